# revision 1
# baseline (speedup 1.0000x reference)
"""Trainium2 Bass kernel for nn_EntitiesIndexingHeadRuleBased (nms_detection).

kernel(**inputs) takes the FULL batch (B=64) and returns (sub_dist, obj_dist),
each [64, 500, 500] float32, matching the reference semantics:

  out_s[r,e] = relu(N) * score_e / (u*A*(d+1)*(s+1))
  N          = u^2 - A*(u - I)        (algebraic form of clip(giou,0)*u*A)
  u, A, I    = union, enclosing area, intersection (ent_e box vs rel box)
  d          = |vx-cx_e| + |vy-cy_e| ;  s = sqrt(cdist^2 via matmul)

Sharding: pure data parallelism - batch 64 split as 8 images per NeuronCore
across 8 cores (SPMD, one Bass program).

Performance notes (v2):
  * fp16 intermediates: tensor_scalar runs in 4x DVE mode, tensor_tensor in
    2x; scalar_tensor_tensor (1x always) is reserved for the few
    cancellation-sensitive ops which compute in the fp32 ALU and emit
    value-scale fp16 (keeps rel-err ~5e-4 despite fp16 storage).
  * box coords are pre-scaled by 1/256 so all giou quantities fit fp16 range.
  * matmul packs are fp16 (1 cyc/row on PE instead of 4 for fp32).
  * a slice of the per-chunk map ops runs on GPSIMD (Pool) to unload DVE.
  * output is fp16 (halves write traffic); host converts to f32.
"""
import sys
sys.path.insert(0, '/opt/trn_rl_repo')

import numpy as np
import bass_rust
import concourse.bass as bass
import concourse.tile as tile
import concourse.tile as tile_mod
from concourse import mybir
from concourse import bass_utils
from concourse.masks import make_identity
from concourse.tile import TileContext

F32 = mybir.dt.float32
F16 = mybir.dt.float16
AF = mybir.ActivationFunctionType
OP = mybir.AluOpType

B = 64
NE = 500
NR = 500
NC1 = 151
NCL = 150
P = 125
NCH = 4
N_CORES = 8
N_IMG = B // N_CORES

SCALE = 1.0 / 256.0          # box-coordinate prescale for fp16 range
D2_BIAS = 3e-4               # clamp for sqrt(d2) against fp16 rounding

# Which map ops run on GPSIMD (Pool) instead of DVE. Walrus only accepts
# plain TensorTensor (add/mult) there - TensorScalarPtr fails engine check.
POOL_OPS = frozenset({"d12", "outv", "Pn", "den", "D3"})

# ---------------------------------------------------------------------------
# Workarounds for the container's walrus: it rejects instructions carrying
# more than one sync-wait command ("Too many sync wait commands").
# ---------------------------------------------------------------------------

_MAXW = 1


def _patched_drain_and_barrier(self, tick_clock, wait_clock):
    ScopedClock = tile_mod.ScopedClock
    carrier = self.nc.sync.nop(nofuse=True)
    wait_clock.add_sem_waits(carrier.ins,
                             ScopedClock({None: tick_clock.global_clock}))
    si = carrier.ins.sync_info
    waits = list(si.on_wait) if si is not None else []
    if len(waits) > _MAXW:
        carrier.ins.sync_info = bass_rust.SyncInfo(
            on_wait=waits[:_MAXW], on_update=[])
        for i in range(_MAXW, len(waits), _MAXW):
            nop = self.nc.sync.nop(nofuse=True)
            nop.ins.sync_info = bass_rust.SyncInfo(
                on_wait=waits[i:i + _MAXW], on_update=[])
    self.nc.sync.drain()
    self.nc.all_engine_barrier()
    assert self.sems is not None
    popped = self.nc._tile_sem_poison_stack.pop()
    assert popped is self._sem_poison
    self.nc.clear_and_free_semaphores(list(self.sems.allocated().values()))
    self.nc.all_engine_barrier()


TileContext._drain_and_barrier = _patched_drain_and_barrier


def _split_waits(nc, maxw=_MAXW):
    """Hoist excess sync waits onto same-engine NoOps placed just before the
    offending instruction (engine streams execute in order)."""
    for fn in nc.m.functions:
        for blk in fn.blocks:
            newl = []
            changed = False
            for ins in blk.instructions:
                si = ins.sync_info
                waits = list(si.on_wait) if si is not None else []
                if len(waits) > maxw:
                    changed = True
                    carried, rest = waits[:-maxw], waits[-maxw:]
                    for i in range(0, len(carried), maxw):
                        nop = mybir.InstNoOp(
                            name=f"{ins.name}-sw{i}",
                            sync_info=mybir.SyncInfo(
                                on_wait=carried[i:i + maxw], on_update=[]),
                            bass_nofuse=True,
                            engine=ins.engine,
                        )
                        newl.append(nop)
                    ins.sync_info = mybir.SyncInfo(
                        on_wait=rest, on_update=list(si.on_update))
                newl.append(ins)
            if changed:
                blk.instructions = newl


# ---------------------------------------------------------------------------
# Kernel builder
# ---------------------------------------------------------------------------

def _bcast(ap, p):
    """[1,N] DRAM AP -> [p,N] partition-broadcast AP (stride-0 partition)."""
    return bass.AP(tensor=ap.tensor, offset=ap.offset,
                   ap=[[0, p]] + list(ap.ap[1:]))


def _build(n_img):
    nc = bass.Bass("TRN2", target_bir_lowering=False)

    pb = nc.dram_tensor("pred_boxes", [n_img, NE, 4], F32, kind="ExternalInput")
    pl = nc.dram_tensor("pred_logits", [n_img, NE, NC1], F32, kind="ExternalInput")
    rol = nc.dram_tensor("pred_rel_obj_logits", [n_img, NR, NC1], F32, kind="ExternalInput")
    rsl = nc.dram_tensor("pred_rel_sub_logits", [n_img, NR, NC1], F32, kind="ExternalInput")
    rob = nc.dram_tensor("pred_rel_obj_box", [n_img, NR, 4], F32, kind="ExternalInput")
    rsb = nc.dram_tensor("pred_rel_sub_box", [n_img, NR, 4], F32, kind="ExternalInput")
    rv = nc.dram_tensor("pred_rel_vec", [n_img, NR, 4], F32, kind="ExternalInput")
    tsz = nc.dram_tensor("target_sizes", [n_img, 2], F32, kind="ExternalInput")
    out_s = nc.dram_tensor("out_sub", [n_img, NR, NE], F16, kind="ExternalOutput")
    out_o = nc.dram_tensor("out_obj", [n_img, NR, NE], F16, kind="ExternalOutput")

    with tile.TileContext(nc) as tc:
        with (
            tc.tile_pool(name="singles", bufs=1) as singles,
            tc.tile_pool(name="io", bufs=3) as io,
            tc.tile_pool(name="pre", bufs=5) as pre,
            tc.tile_pool(name="col", bufs=2) as col,
            tc.tile_pool(name="rep", bufs=2) as rep,
            tc.tile_pool(name="mm", bufs=2) as mm,
            tc.tile_pool(name="mp", bufs=2) as mp,
            tc.tile_pool(name="ps", bufs=2, space="PSUM") as ps,
            tc.tile_pool(name="psd", bufs=4, space="PSUM") as psd,
            tc.tile_pool(name="dr", bufs=2, space="DRAM") as dr,
        ):
            ident = singles.tile([128, 128], F32, tag="ident")
            make_identity(nc, ident)
            ident16 = singles.tile([128, 128], F16, tag="ident16")
            nc.vector.tensor_copy(out=ident16, in_=ident)
            d2b = singles.tile([128, 1], F32, tag="d2b")
            nc.vector.memset(d2b, D2_BIAS)

            # image-level software pipeline: image b+1's prep (softmax,
            # boxes, broadcasts) is emitted in the middle of image b's map
            # stage so its ACT/DVE/DMA work hides in the map stage's slack.
            env = locals()
            ctx = _prep_image(nc, 0, env)
            for b in range(n_img):
                holder = []
                if b + 1 < n_img:
                    def mid(b=b, holder=holder):
                        holder.append(_prep_image(nc, b + 1, env))
                else:
                    def mid():
                        return None
                _maps_image(nc, b, env, ctx, mid)
                ctx = holder[0] if holder else None
    _split_waits(nc)
    return nc


def _prep_image(nc, b, env):
    io, pre, col, rep, mm, mp, ps, psd, dr = (env[k] for k in
        ("io", "pre", "col", "rep", "mm", "mp", "ps", "psd", "dr"))
    ident = env["ident"]
    ident16 = env["ident16"]
    d2b = env["d2b"]
    pb, pl, rol, rsl, rob, rsb, rv, tsz = (env[k] for k in
        ("pb", "pl", "rol", "rsl", "rob", "rsb", "rv", "tsz"))
    out_s, out_o = env["out_s"], env["out_o"]

    # image-level scalars: W, H broadcast to all partitions (px and /256)
    WH = col.tile([128, 2], F32, tag="WH")
    nc.sync.dma_start(out=WH, in_=_bcast(tsz[b:b + 1, :], 128))
    Ht = WH[:, 0:1]
    Wt = WH[:, 1:2]
    HtP = WH[:P, 0:1]
    WtP = WH[:P, 1:2]
    Ws = col.tile([128, 1], F32, tag="Ws")
    Hs = col.tile([128, 1], F32, tag="Hs")
    nWs = col.tile([128, 1], F32, tag="nWs")
    nHs = col.tile([128, 1], F32, tag="nHs")
    nc.vector.tensor_scalar(out=Ws, in0=Wt, scalar1=SCALE, scalar2=None,
                            op0=OP.mult)
    nc.vector.tensor_scalar(out=Hs, in0=Ht, scalar1=SCALE, scalar2=None,
                            op0=OP.mult)
    nc.vector.tensor_scalar(out=nWs, in0=Wt, scalar1=-SCALE, scalar2=None,
                            op0=OP.mult)
    nc.vector.tensor_scalar(out=nHs, in0=Ht, scalar1=-SCALE, scalar2=None,
                            op0=OP.mult)

    # --- batched box prep: ent / rs / ro processed in one [125,12,*] pass ---
    # columns of PIXA: 0 x0s 1 y0s 2 nx1s 3 ny1s 4 ws 5 hs 6 areas
    BTA = io.tile([P, 3, NCH, 4], F32, tag="BTA")
    for t, dram in enumerate((pb, rsb, rob)):
        nc.sync.dma_start(out=BTA[:, t, :, :],
                          in_=dram[b].rearrange("(j p) c -> p j c", p=P))
    BTF = BTA[:, :, :, :].rearrange("p t j c -> p (t j) c")
    LO = col.tile([P, 12, 2], F32, tag="LO")
    HIc = col.tile([P, 12, 2], F32, tag="HIc")
    nc.vector.scalar_tensor_tensor(
        out=LO, in0=BTF[:, :, 2:4], scalar=-0.5, in1=BTF[:, :, 0:2],
        op0=OP.mult, op1=OP.add)
    nc.vector.scalar_tensor_tensor(
        out=HIc, in0=BTF[:, :, 2:4], scalar=0.5, in1=BTF[:, :, 0:2],
        op0=OP.mult, op1=OP.add)
    PIXA = col.tile([P, 3, NCH, 7], F32, tag="PIXA")
    PIXF = PIXA[:, :, :, :].rearrange("p t j c -> p (t j) c")
    nc.vector.tensor_scalar(out=PIXF[:, :, 0:1], in0=LO[:, :, 0:1],
                            scalar1=Ws[:P], scalar2=None, op0=OP.mult)
    nc.vector.tensor_scalar(out=PIXF[:, :, 1:2], in0=LO[:, :, 1:2],
                            scalar1=Hs[:P], scalar2=None, op0=OP.mult)
    nc.vector.tensor_scalar(out=PIXF[:, :, 2:3], in0=HIc[:, :, 0:1],
                            scalar1=nWs[:P], scalar2=None, op0=OP.mult)
    nc.vector.tensor_scalar(out=PIXF[:, :, 3:4], in0=HIc[:, :, 1:2],
                            scalar1=nHs[:P], scalar2=None, op0=OP.mult)
    nc.vector.tensor_scalar(out=PIXF[:, :, 4:5], in0=BTF[:, :, 2:3],
                            scalar1=Ws[:P], scalar2=None, op0=OP.mult)
    nc.vector.tensor_scalar(out=PIXF[:, :, 5:6], in0=BTF[:, :, 3:4],
                            scalar1=Hs[:P], scalar2=None, op0=OP.mult)
    nc.vector.tensor_tensor(out=PIXF[:, :, 6:7], in0=PIXF[:, :, 4:5],
                            in1=PIXF[:, :, 5:6], op=OP.mult)
    PIX_E = PIXA[:, 0, :, :]
    PIX_S = PIXA[:, 1, :, :]
    PIX_O = PIXA[:, 2, :, :]

    # PG columns (f32): 0 ws 1 hs 2 areas 3 score | 4 x0s 5 y0s 6 nx1s 7 ny1s
    #                   8 cx_px 9 cy_px   (fp16-bound rows first: partition-0
    #                   aligned reads after the PE transpose)
    PG = pre.tile([P, NCH, 10], F32, tag="PG")
    nc.vector.tensor_copy(out=PG[:, :, 0:3], in_=PIX_E[:, :, 4:7])
    nc.vector.tensor_copy(out=PG[:, :, 4:8], in_=PIX_E[:, :, 0:4])
    nc.vector.tensor_scalar(out=PG[:, :, 8:9], in0=BTA[:, 0, :, 0:1],
                            scalar1=WtP, scalar2=None, op0=OP.mult)
    nc.vector.tensor_scalar(out=PG[:, :, 9:10], in0=BTA[:, 0, :, 1:2],
                            scalar1=HtP, scalar2=None, op0=OP.mult)

    # rel_vec endpoints, negated, px units (ACT bias for |cx - vx|).
    # WH4 = [w,h,w,h] per partition via a reversed-stride broadcast DMA of
    # target_sizes ([h,w] in DRAM); VN = (-rel_vec) * WH4 in one op.
    RVt = io.tile([P, NCH, 4], F32, tag="RVt")
    nc.sync.dma_start(out=RVt, in_=rv[b].rearrange("(j p) c -> p j c", p=P))
    WH4 = col.tile([128, 4], F32, tag="WH4")
    _wh = WH[:, :]
    nc.vector.tensor_copy(out=WH4, in_=bass.AP(
        tensor=_wh.tensor, offset=_wh.offset + 1,
        ap=[list(_wh.ap[0]), [0, 2], [-1, 2]]))
    VN = col.tile([P, NCH, 4], F32, tag="VN")
    _w4 = WH4[:P]
    nc.vector.scalar_tensor_tensor(
        out=VN, in0=RVt, scalar=-1.0,
        in1=bass.AP(tensor=_w4.tensor, offset=_w4.offset,
                    ap=[list(_w4.ap[0]), [0, NCH]] + list(_w4.ap[1:])),
        op0=OP.mult, op1=OP.mult)

    # softmax + fp16 packs + PE transposes into class-major matmul operands
    RHS_A = mm.tile([128, NE], F16, tag="RHS_A")
    RHS_B = mm.tile([24, NE], F16, tag="RHS_B")
    LS_A = mm.tile([128, NR], F16, tag="LS_A")
    LS_B = mm.tile([24, NR], F16, tag="LS_B")
    LO_A = mm.tile([128, NR], F16, tag="LO_A")
    LO_B = mm.tile([24, NR], F16, tag="LO_B")

    for t, (ldram, dA, dB) in {
        "ent": (pl, RHS_A, RHS_B),
        "rs": (rsl, LS_A, LS_B),
        "ro": (rol, LO_A, LO_B),
    }.items():
        LT = io.tile([P, NCH, NC1], F32, tag="LT")
        nc.sync.dma_start(out=LT, in_=ldram[b].rearrange("(j p) c -> p j c",
                                                         p=P))
        # per-chunk exp/square accumulators land in [P, NCH] column tiles so
        # the tiny scalar algebra runs once per image, not once per chunk
        E4 = pre.tile([P, NCH, NC1], F16, tag="E4")
        SM = col.tile([P, NCH, 1], F32, tag="SM")
        SQC = col.tile([P, NCH, 1], F32, tag="SQC")
        for j in range(NCH):
            nc.scalar.activation(out=E4[:, j, :], in_=LT[:, j, :],
                                 func=AF.Exp, accum_out=SM[:, j, :])
            SQ = pre.tile([P, NCL], F16, tag="SQ")
            nc.scalar.activation(out=SQ, in_=E4[:, j, :NCL], func=AF.Square,
                                 accum_out=SQC[:, j, :])
        R4 = col.tile([P, NCH, 1], F32, tag="R4")
        nc.vector.reciprocal(R4, SM)
        pk_s2 = None if t == "ent" else -2.0
        PKs = []
        for j in range(NCH):
            PK = pre.tile([P, 152], F16, tag="PK")
            PKs.append(PK)
            if pk_s2 is None:
                nc.vector.tensor_scalar(out=PK[:, 0:NCL], in0=E4[:, j, :NCL],
                                        scalar1=R4[:, j, :], scalar2=None,
                                        op0=OP.mult)
            else:
                nc.vector.tensor_scalar(out=PK[:, 0:NCL], in0=E4[:, j, :NCL],
                                        scalar1=R4[:, j, :], scalar2=pk_s2,
                                        op0=OP.mult, op1=OP.mult)
        r2 = col.tile([P, NCH, 1], F32, tag="r2")
        nc.vector.tensor_tensor(out=r2, in0=R4, in1=R4, op=OP.mult)
        if t == "ent":
            MX = col.tile([P, NCH, 1], F32, tag="MX")
            nc.vector.tensor_reduce(out=MX, in_=E4[:, :, :NCL],
                                    axis=mybir.AxisListType.X, op=OP.max)
            PK151 = col.tile([P, NCH, 1], F32, tag="PK151")
            nc.vector.tensor_tensor(out=PK151, in0=SQC, in1=r2, op=OP.mult)
            nc.vector.tensor_tensor(out=PG[:, :, 3:4], in0=MX, in1=R4,
                                    op=OP.mult)
            k150, k151 = 0.25, PK151
        else:
            PK150 = col.tile([P, NCH, 1], F32, tag="PK150")
            nc.vector.scalar_tensor_tensor(out=PK150, in0=r2, scalar=4.0,
                                           in1=SQC, op0=OP.mult, op1=OP.mult)
            k150, k151 = PK150, 1.0
        for j in range(NCH):
            PK = PKs[j]
            if t == "ent":
                nc.vector.memset(PK[:, 150:151], k150)
                nc.vector.tensor_copy(out=PK[:, 151:152], in_=k151[:, j, :])
            else:
                nc.vector.tensor_copy(out=PK[:, 150:151], in_=k150[:, j, :])
                nc.vector.memset(PK[:, 151:152], k151)
            TA = ps.tile([128, P], F16, tag="TT16")
            nc.tensor.transpose(TA, PK[:, 0:128], ident16[:P, :P])
            nc.scalar.copy(out=dA[:, P * j:P * (j + 1)], in_=TA)
            TB = ps.tile([24, P], F16, tag="TT16")
            nc.tensor.transpose(TB, PK[:, 128:152], ident16[:P, :P])
            nc.scalar.copy(out=dB[:, P * j:P * (j + 1)], in_=TB)

    # entity-side rows -> DRAM -> broadcast into replicated tiles.
    # EROWS rows: 0 x0s 1 y0s 2 nx1s 3 ny1s 4 cx 5 cy | 6 ws 7 hs 8 areas 9 scr
    # rows 0..5 stay f32 (one [P,6,NE] broadcast); rows 6..9 are converted to
    # fp16 and broadcast DOUBLED ([P,4,2,NE]) for the fused sub/obj map ops.
    EROWS = rep.tile([10, NE], F32, tag="EROWS")
    for j in range(NCH):
        TE = ps.tile([10, P], F32, tag="TT")
        nc.tensor.transpose(TE, PG[:, j, :], ident[:P, :P])
        nc.scalar.copy(out=EROWS[:, P * j:P * (j + 1)], in_=TE)
    EROWS16 = rep.tile([4, NE], F16, tag="EROWS16")
    nc.vector.tensor_copy(out=EROWS16, in_=EROWS[0:4, :])
    ED32 = dr.tile([6, NE], F32, tag="ED32", name="ED32")
    nc.sync.dma_start(out=ED32, in_=EROWS[4:10, :])
    ED16 = dr.tile([4, NE], F16, tag="ED16", name="ED16")
    nc.sync.dma_start(out=ED16, in_=EROWS16)
    RALL = rep.tile([P, 6, NE], F32, tag="RALL", name="RALL")
    _ed = ED32[:, :]
    nc.sync.dma_start(out=RALL, in_=bass.AP(
        tensor=_ed.tensor, offset=_ed.offset,
        ap=[[0, P]] + list(_ed.ap)))
    RD4 = rep.tile([P, 4, 2, NE], F16, tag="RD4", name="RD4")
    _eh = ED16[:, :]
    for k in range(2):
        nc.sync.dma_start(out=RD4[:, :, k, :], in_=bass.AP(
            tensor=_eh.tensor, offset=_eh.offset,
            ap=[[0, P]] + list(_eh.ap)))
    R = {}
    for k, nm_ in enumerate(["X0R", "Y0R", "X1N", "Y1N", "CXR", "CYR"]):
        R[nm_] = RALL[:, k, :]
    for k, nm_ in enumerate(["WER2", "HER2", "AREAR2", "SCR2"]):
        R[nm_] = RD4[:, k, :, :]
    return dict(R=R, VN=VN, PIX_S=PIX_S, PIX_O=PIX_O, PIXA=PIXA, RALL=RALL,
                RD4=RD4,
                RHS_A=RHS_A, RHS_B=RHS_B, LS_A=LS_A, LS_B=LS_B,
                LO_A=LO_A, LO_B=LO_B)


def _maps_image(nc, b, env, ctx, mid_thunk):
    io, pre, col, rep, mm, mp, ps, psd, dr = (env[k] for k in
        ("io", "pre", "col", "rep", "mm", "mp", "ps", "psd", "dr"))
    d2b = env["d2b"]
    out_s, out_o = env["out_s"], env["out_o"]
    R = ctx["R"]
    VN = ctx["VN"]
    PIXA = ctx["PIXA"]
    RALL = ctx["RALL"]
    RD4 = ctx["RD4"]
    RHS_A, RHS_B = ctx["RHS_A"], ctx["RHS_B"]

    # fused sub/obj map pipeline: per chunk j, both maps are computed in
    # [P, 2, NE] pair tiles; ops without map-specific scalars run once over
    # the doubled free dim.
    MAPS = ((ctx["LS_A"], ctx["LS_B"], ctx["PIX_S"], 0, 1, out_s),
            (ctx["LO_A"], ctx["LO_B"], ctx["PIX_O"], 2, 3, out_o))

    def m2(tag, dt=F16):
        return mp.tile([P, 2, NE], dt, tag=tag, name=tag)

    def eng(name):
        return nc.gpsimd if name in POOL_OPS else nc.vector

    def stageA_make(j):
        """Allocate pair-j stage-A tiles; return (tiles, dve_thunks,
        pe_thunks, act_thunks). DVE thunks are ordered so no op reads a tile
        written by the immediately-preceding DVE op (write-to-read stall)."""
        ax_p, ay_p, s_p = m2("ax"), m2("ay"), m2("s")
        dxn = m2("dxn")
        dyn = m2("dyn")
        vx1n = [mp.tile([P, NE], F32, tag="vx1n", name="vx1n")
                for _ in range(2)]
        vy1n = [mp.tile([P, NE], F32, tag="vy1n", name="vy1n")
                for _ in range(2)]
        D2s = [psd.tile([P, NE], F32, tag="D2", name="D2") for _ in range(2)]
        pe, act, dve = [], [], []
        for mi, (lA, lB, PIXR, vxc, vyc, odram) in enumerate(MAPS):
            def _mm(mi=mi, lA=lA, lB=lB):
                nc.tensor.matmul(D2s[mi], lhsT=lA[:, P * j:P * (j + 1)],
                                 rhs=RHS_A, start=True, stop=False)
                nc.tensor.matmul(D2s[mi], lhsT=lB[:, P * j:P * (j + 1)],
                                 rhs=RHS_B, start=False, stop=True)
            pe.append(_mm)
            act.append(lambda mi=mi, vxc=vxc: nc.scalar.activation(
                out=ax_p[:, mi, :], in_=R["CXR"], func=AF.Abs,
                bias=VN[:, j, vxc:vxc + 1]))
            act.append(lambda mi=mi, vyc=vyc: nc.scalar.activation(
                out=ay_p[:, mi, :], in_=R["CYR"], func=AF.Abs,
                bias=VN[:, j, vyc:vyc + 1]))
            dve.append(lambda mi=mi, PIXR=PIXR: nc.vector.tensor_scalar(
                out=vx1n[mi], in0=R["X1N"], scalar1=PIXR[:, j, 2:3],
                scalar2=None, op0=OP.max))
            dve.append(lambda mi=mi, PIXR=PIXR: nc.vector.tensor_scalar(
                out=vy1n[mi], in0=R["Y1N"], scalar1=PIXR[:, j, 3:4],
                scalar2=None, op0=OP.max))
        for mi, (lA, lB, PIXR, vxc, vyc, odram) in enumerate(MAPS):
            dve.append(lambda mi=mi, PIXR=PIXR: nc.vector.scalar_tensor_tensor(
                out=dxn[:, mi, :], in0=R["X0R"], scalar=PIXR[:, j, 0:1],
                in1=vx1n[mi], op0=OP.max, op1=OP.add))
            dve.append(lambda mi=mi, PIXR=PIXR: nc.vector.scalar_tensor_tensor(
                out=dyn[:, mi, :], in0=R["Y0R"], scalar=PIXR[:, j, 1:2],
                in1=vy1n[mi], op0=OP.max, op1=OP.add))
        for mi in range(2):
            act.append(lambda mi=mi: nc.scalar.activation(
                out=s_p[:, mi, :], in_=D2s[mi], func=AF.Sqrt, bias=d2b[:P]))
        return (ax_p, ay_p, s_p, dxn, dyn), dve, pe, act

    def fusedStage(j, stA, filler, tail_thunks):
        ax_p, ay_p, s_p, dxn, dyn = stA

        def F():
            for th in filler:
                th()
                return

        def ts(name, in0, s1, op0, s2=None, op1=None, dt=F16, out=None):
            o = out if out is not None else m2(name, dt)
            tgt = o if out is None else out
            if op1 is None:
                eng(name).tensor_scalar(out=tgt, in0=in0, scalar1=s1,
                                        scalar2=None, op0=op0)
            else:
                eng(name).tensor_scalar(out=tgt, in0=in0, scalar1=s1,
                                        scalar2=s2, op0=op0, op1=op1)
            return o

        def tt(name, in0, in1, op, dt=F16):
            o = m2(name, dt)
            eng(name).tensor_tensor(out=o, in0=in0, in1=in1, op=op)
            return o

        PIX0 = MAPS[0][2]
        PIX1 = MAPS[1][2]
        # ---- fused stage; F() = one stage-A(j+1) DVE op as a gap spacer ----
        i1 = m2("i1")
        nc.vector.scalar_tensor_tensor(                        # relu(dx)*dy
            out=i1, in0=dxn, scalar=0.0, in1=dyn, op0=OP.min, op1=OP.mult)
        d12 = tt("d12", ax_p, ay_p, OP.add)                    # pool
        F()
        I2 = m2("I2")
        w1 = m2("w1")
        h1 = m2("h1")
        ts("I2", i1[:, 0, :], 0.0, OP.max, PIX0[:, j, 6:7], OP.subtract,
           out=I2[:, 0, :])
        ts("w1", dxn[:, 0, :], PIX0[:, j, 4:5], OP.add, out=w1[:, 0, :])
        F()
        ts("I2", i1[:, 1, :], 0.0, OP.max, PIX1[:, j, 6:7], OP.subtract,
           out=I2[:, 1, :])
        ts("w1", dxn[:, 1, :], PIX1[:, j, 4:5], OP.add, out=w1[:, 1, :])
        F()
        U = tt("U", R["AREAR2"], I2, OP.subtract)              # union
        ts("h1", dyn[:, 0, :], PIX0[:, j, 5:6], OP.add, out=h1[:, 0, :])
        ts("h1", dyn[:, 1, :], PIX1[:, j, 5:6], OP.add, out=h1[:, 1, :])
        F()
        mI = m2("mI")
        nc.vector.scalar_tensor_tensor(                        # I - u
            out=mI, in0=i1, scalar=0.0, in1=U, op0=OP.max, op1=OP.subtract)
        sq = m2("sq")
        nc.scalar.activation(out=sq, in_=U, func=AF.Square)    # u^2
        wc = tt("wc", w1, R["WER2"], OP.add)
        F()
        hc = tt("hc", h1, R["HER2"], OP.add)
        F()
        A = tt("A", wc, hc, OP.mult)                           # areac
        s1 = ts("s1", s_p, 1.0, OP.add)
        F()
        prod = tt("prod", A, mI, OP.mult)
        Pn = tt("Pn", U, A, OP.mult)                           # pool
        ds1 = ts("ds1", d12, 1.0, OP.add)
        F()
        N = tt("N", sq, prod, OP.add)                          # numerator
        den = tt("den", s1, ds1, OP.mult)                      # pool
        D3 = tt("D3", Pn, den, OP.mult, dt=F32)                # pool, > 0
        # 1/D3 on ScalarE via exp(-ln(D3)) - keeps the (slow, iterative)
        # DVE reciprocal off the bottleneck engine.
        lg = m2("lg")
        nc.scalar.activation(out=lg, in_=D3, func=AF.Ln)
        r3 = m2("r3", F32)
        nc.scalar.activation(out=r3, in_=lg, func=AF.Exp, scale=-1.0)
        F()
        nm = m2("nm")
        nc.vector.scalar_tensor_tensor(out=nm, in0=N, scalar=0.0,
                                       in1=r3, op0=OP.max, op1=OP.mult)
        for th in tail_thunks:
            th()
        outv = tt("outv", nm, R["SCR2"], OP.mult)              # pool
        for mi, (lA, lB, PIXR, vxc, vyc, odram) in enumerate(MAPS):
            nc.sync.dma_start(out=odram[b, P * j:P * (j + 1), :],
                              in_=outv[:, mi, :])
        for th in filler:
            th()

    # depth-2 software pipeline with fine-grained interleave: pair j+1's
    # independent DVE ops are woven between pair j's dependent ops so the
    # DVE never reads a tile written by its immediately-preceding op.
    stA, dveA, peA, actA = stageA_make(0)
    for th in peA + actA + dveA:
        th()
    for j in range(NCH):
        if j + 1 < NCH:
            nxt, dveN, peN, actN = stageA_make(j + 1)
            for th in peN:
                th()
        else:
            nxt, dveN, actN = None, [], []
        fusedStage(j, stA, iter(dveN), actN)
        stA = nxt
        if j == 0:
            mid_thunk()  # next image prep hides in this image's map slack


class _CompiledKernel:
    """Compiled SPMD executable: jit built once, reusable across calls."""

    def __init__(self, nc, n_cores):
        import jax
        from jax.sharding import Mesh, PartitionSpec
        try:
            from jax.experimental.shard_map import shard_map
        except Exception:
            from jax.shard_map import shard_map
        from concourse import bass2jax
        from concourse.bass2jax import _bass_exec_p, install_neuronx_cc_hook

        install_neuronx_cc_hook()
        self.jax = jax
        self.n_cores = n_cores
        partition_name = (nc.partition_id_tensor.name
                          if nc.partition_id_tensor else None)
        in_names, out_names, out_avals, zero_outs = [], [], [], []
        for alloc in nc.m.functions[0].allocations:
            if not isinstance(alloc, mybir.MemoryLocationSet):
                continue
            name = alloc.memorylocations[0].name
            if alloc.kind == "ExternalInput":
                if name != partition_name:
                    in_names.append(name)
            elif alloc.kind == "ExternalOutput":
                shape = tuple(alloc.tensor_shape)
                dtype = mybir.dt.np(alloc.dtype)
                out_names.append(name)
                out_avals.append(jax.core.ShapedArray(shape, dtype))
                zero_outs.append(np.zeros(shape, dtype))
        self.in_names = in_names
        self.out_names = out_names
        self.out_avals = out_avals
        self.zero_outs = zero_outs
        all_in = in_names + out_names
        if partition_name is not None:
            all_in.append(partition_name)

        def _body(*args):
            operands = list(args)
            if partition_name is not None:
                operands.append(bass2jax.partition_id_tensor())
            return tuple(_bass_exec_p.bind(
                *operands,
                out_avals=tuple(out_avals),
                in_names=tuple(all_in),
                out_names=tuple(out_names),
                lowering_input_output_aliases=(),
                sim_require_finite=True,
                sim_require_nnan=True,
                nc=nc,
            ))

        devices = jax.devices()[:n_cores]
        self._mesh = Mesh(np.asarray(devices), ("core",))
        nin = len(in_names) + len(out_names)
        sm = shard_map(_body, mesh=self._mesh,
                       in_specs=(PartitionSpec("core"),) * nin,
                       out_specs=(PartitionSpec("core"),) * len(out_names),
                       check_rep=False)
        from jax.sharding import NamedSharding
        sh = NamedSharding(self._mesh, PartitionSpec("core"))
        in_abst = []
        for alloc in nc.m.functions[0].allocations:
            if not isinstance(alloc, mybir.MemoryLocationSet):
                continue
            name = alloc.memorylocations[0].name
            if alloc.kind == "ExternalInput" and name in in_names:
                shape = tuple(alloc.tensor_shape)
                in_abst.append(jax.ShapeDtypeStruct(
                    (n_cores * shape[0], *shape[1:]), mybir.dt.np(alloc.dtype),
                    sharding=sh))
        out_abst = [jax.ShapeDtypeStruct((n_cores * z.shape[0], *z.shape[1:]),
                                         z.dtype, sharding=sh)
                    for z in self.zero_outs]
        try:
            from concourse.bass2jax import fast_dispatch_compile
            self._fn = fast_dispatch_compile(
                lambda: jax.jit(sm, keep_unused=True)
                .lower(*in_abst, *out_abst).compile())
        except Exception:
            self._fn = jax.jit(sm, keep_unused=True)

    def run(self, in_maps):
        jax = self.jax
        n = self.n_cores
        per_core = [[np.asarray(m[nm]) for nm in self.in_names]
                    for m in in_maps]
        concat_in = [np.concatenate([per_core[c][i] for c in range(n)], axis=0)
                     for i in range(len(self.in_names))]
        concat_zero = [np.zeros((n * z.shape[0], *z.shape[1:]), z.dtype)
                       for z in self.zero_outs]
        outs = jax.block_until_ready(self._fn(*concat_in, *concat_zero))
        return [
            {nm: np.asarray(outs[i]).reshape(n, *self.out_avals[i].shape)[c]
             for i, nm in enumerate(self.out_names)}
            for c in range(n)
        ]


_CACHE = {}


def _get_nc():
    if "nc" not in _CACHE:
        _CACHE["nc"] = _build(N_IMG)
    return _CACHE["nc"]


def _get_ck():
    if "ck" not in _CACHE:
        _CACHE["ck"] = _CompiledKernel(_get_nc(), N_CORES)
    return _CACHE["ck"]


def kernel(pred_boxes, pred_logits, pred_rel_obj_logits, pred_rel_sub_logits,
           pred_rel_obj_box, pred_rel_sub_box, pred_rel_vec, target_sizes):
    inp = {
        "pred_boxes": np.ascontiguousarray(pred_boxes, dtype=np.float32),
        "pred_logits": np.ascontiguousarray(pred_logits, dtype=np.float32),
        "pred_rel_obj_logits": np.ascontiguousarray(pred_rel_obj_logits, dtype=np.float32),
        "pred_rel_sub_logits": np.ascontiguousarray(pred_rel_sub_logits, dtype=np.float32),
        "pred_rel_obj_box": np.ascontiguousarray(pred_rel_obj_box, dtype=np.float32),
        "pred_rel_sub_box": np.ascontiguousarray(pred_rel_sub_box, dtype=np.float32),
        "pred_rel_vec": np.ascontiguousarray(pred_rel_vec, dtype=np.float32),
        "target_sizes": np.ascontiguousarray(target_sizes, dtype=np.float32),
    }
    in_maps = [{k: v[c * N_IMG:(c + 1) * N_IMG] for k, v in inp.items()}
               for c in range(N_CORES)]
    res = None
    try:
        res = _get_ck().run(in_maps)
    except Exception:
        import time as _time
        _time.sleep(2.0)
        try:
            res = _get_ck().run(in_maps)
        except Exception:
            r = bass_utils.run_bass_kernel_spmd(
                _get_nc(), in_maps, core_ids=list(range(N_CORES)))
            res = r.results
    sub = np.concatenate([res[c]["out_sub"] for c in range(N_CORES)], axis=0)
    obj = np.concatenate([res[c]["out_obj"] for c in range(N_CORES)], axis=0)
    return np.float32(sub), np.float32(obj)



# revision 4
# speedup vs baseline: 7.1684x; 7.1684x over previous
"""Trainium2 Bass kernel for nn_EntitiesIndexingHeadRuleBased (nms_detection).

kernel(**inputs) takes the FULL batch (B=64) and returns (sub_dist, obj_dist),
each [64, 500, 500] float32, matching the reference semantics:

  out_s[r,e] = relu(N) * score_e / (u*A*(d+1)*(s+1))
  N          = u^2 - A*(u - I)        (algebraic form of clip(giou,0)*u*A)
  u, A, I    = union, enclosing area, intersection (ent_e box vs rel box)
  d          = |vx-cx_e| + |vy-cy_e| ;  s = sqrt(cdist^2 via matmul)

Sharding: pure data parallelism - batch 64 split as 8 images per NeuronCore
across 8 cores (SPMD, one Bass program).

Performance notes (v2):
  * fp16 intermediates: tensor_scalar runs in 4x DVE mode, tensor_tensor in
    2x; scalar_tensor_tensor (1x always) is reserved for the few
    cancellation-sensitive ops which compute in the fp32 ALU and emit
    value-scale fp16 (keeps rel-err ~5e-4 despite fp16 storage).
  * box coords are pre-scaled by 1/256 so all giou quantities fit fp16 range.
  * matmul packs are fp16 (1 cyc/row on PE instead of 4 for fp32).
  * a slice of the per-chunk map ops runs on GPSIMD (Pool) to unload DVE.
  * output is fp16 (halves write traffic); host converts to f32.
"""
import sys
sys.path.insert(0, '/opt/trn_rl_repo')

import numpy as np
import bass_rust
import concourse.bass as bass
import concourse.tile as tile
import concourse.tile as tile_mod
from concourse import mybir
from concourse import bass_utils
from concourse.masks import make_identity
from concourse.tile import TileContext

F32 = mybir.dt.float32
F16 = mybir.dt.float16
AF = mybir.ActivationFunctionType
OP = mybir.AluOpType

B = 64
NE = 500
NR = 500
NC1 = 151
NCL = 150
P = 125
NCH = 4
N_CORES = 8
N_IMG = B // N_CORES

SCALE = 1.0 / 256.0          # box-coordinate prescale for fp16 range
D2_BIAS = 3e-4               # clamp for sqrt(d2) against fp16 rounding

# Which map ops run on GPSIMD (Pool) instead of DVE. Walrus only accepts
# plain TensorTensor (add/mult) there - TensorScalarPtr fails engine check.
POOL_OPS = frozenset({"d12", "outv", "Pn", "den", "D3"})

# ---------------------------------------------------------------------------
# Workarounds for the container's walrus: it rejects instructions carrying
# more than one sync-wait command ("Too many sync wait commands").
# ---------------------------------------------------------------------------

_MAXW = 1


def _patched_drain_and_barrier(self, tick_clock, wait_clock):
    ScopedClock = tile_mod.ScopedClock
    carrier = self.nc.sync.nop(nofuse=True)
    wait_clock.add_sem_waits(carrier.ins,
                             ScopedClock({None: tick_clock.global_clock}))
    si = carrier.ins.sync_info
    waits = list(si.on_wait) if si is not None else []
    if len(waits) > _MAXW:
        carrier.ins.sync_info = bass_rust.SyncInfo(
            on_wait=waits[:_MAXW], on_update=[])
        for i in range(_MAXW, len(waits), _MAXW):
            nop = self.nc.sync.nop(nofuse=True)
            nop.ins.sync_info = bass_rust.SyncInfo(
                on_wait=waits[i:i + _MAXW], on_update=[])
    self.nc.sync.drain()
    self.nc.all_engine_barrier()
    assert self.sems is not None
    popped = self.nc._tile_sem_poison_stack.pop()
    assert popped is self._sem_poison
    self.nc.clear_and_free_semaphores(list(self.sems.allocated().values()))
    self.nc.all_engine_barrier()


TileContext._drain_and_barrier = _patched_drain_and_barrier


def _split_waits(nc, maxw=_MAXW):
    """Hoist excess sync waits onto same-engine NoOps placed just before the
    offending instruction (engine streams execute in order)."""
    for fn in nc.m.functions:
        for blk in fn.blocks:
            newl = []
            changed = False
            for ins in blk.instructions:
                si = ins.sync_info
                waits = list(si.on_wait) if si is not None else []
                if len(waits) > maxw:
                    changed = True
                    carried, rest = waits[:-maxw], waits[-maxw:]
                    for i in range(0, len(carried), maxw):
                        nop = mybir.InstNoOp(
                            name=f"{ins.name}-sw{i}",
                            sync_info=mybir.SyncInfo(
                                on_wait=carried[i:i + maxw], on_update=[]),
                            bass_nofuse=True,
                            engine=ins.engine,
                        )
                        newl.append(nop)
                    ins.sync_info = mybir.SyncInfo(
                        on_wait=rest, on_update=list(si.on_update))
                newl.append(ins)
            if changed:
                blk.instructions = newl


# ---------------------------------------------------------------------------
# Kernel builder
# ---------------------------------------------------------------------------

def _bcast(ap, p):
    """[1,N] DRAM AP -> [p,N] partition-broadcast AP (stride-0 partition)."""
    return bass.AP(tensor=ap.tensor, offset=ap.offset,
                   ap=[[0, p]] + list(ap.ap[1:]))


def _build(n_img, reps=1):
    nc = bass.Bass("TRN2", target_bir_lowering=False)

    pb = nc.dram_tensor("pred_boxes", [n_img, NE, 4], F32, kind="ExternalInput")
    pl = nc.dram_tensor("pred_logits", [n_img, NE, NC1], F32, kind="ExternalInput")
    rol = nc.dram_tensor("pred_rel_obj_logits", [n_img, NR, NC1], F32, kind="ExternalInput")
    rsl = nc.dram_tensor("pred_rel_sub_logits", [n_img, NR, NC1], F32, kind="ExternalInput")
    rob = nc.dram_tensor("pred_rel_obj_box", [n_img, NR, 4], F32, kind="ExternalInput")
    rsb = nc.dram_tensor("pred_rel_sub_box", [n_img, NR, 4], F32, kind="ExternalInput")
    rv = nc.dram_tensor("pred_rel_vec", [n_img, NR, 4], F32, kind="ExternalInput")
    tsz = nc.dram_tensor("target_sizes", [n_img, 2], F32, kind="ExternalInput")
    out_s = nc.dram_tensor("out_sub", [n_img, NR, NE], F16, kind="ExternalOutput")
    out_o = nc.dram_tensor("out_obj", [n_img, NR, NE], F16, kind="ExternalOutput")

    with tile.TileContext(nc) as tc:
        with (
            tc.tile_pool(name="singles", bufs=1) as singles,
            tc.tile_pool(name="io", bufs=3) as io,
            tc.tile_pool(name="pre", bufs=5) as pre,
            tc.tile_pool(name="col", bufs=2) as col,
            tc.tile_pool(name="rep", bufs=2) as rep,
            tc.tile_pool(name="mm", bufs=2) as mm,
            tc.tile_pool(name="mp", bufs=2) as mp,
            tc.tile_pool(name="ps", bufs=2, space="PSUM") as ps,
            tc.tile_pool(name="psd", bufs=4, space="PSUM") as psd,
            tc.tile_pool(name="dr", bufs=2, space="DRAM") as dr,
        ):
            ident = singles.tile([128, 128], F32, tag="ident")
            make_identity(nc, ident)
            ident16 = singles.tile([128, 128], F16, tag="ident16")
            nc.vector.tensor_copy(out=ident16, in_=ident)
            d2b = singles.tile([128, 1], F32, tag="d2b")
            nc.vector.memset(d2b, D2_BIAS)

            # image-level software pipeline: image b+1's prep (softmax,
            # boxes, broadcasts) is emitted in the middle of image b's map
            # stage so its ACT/DVE/DMA work hides in the map stage's slack.
            # reps>1 repeats the whole batch back-to-back (timing variant);
            # the pipeline runs straight through the seam.
            env = locals()
            seq = [i % n_img for i in range(n_img * reps)]
            ctx = _prep_image(nc, seq[0], env)
            for k, b in enumerate(seq):
                holder = []
                if k + 1 < len(seq):
                    def mid(bn=seq[k + 1], holder=holder):
                        holder.append(_prep_image(nc, bn, env))
                else:
                    def mid():
                        return None
                _maps_image(nc, b, env, ctx, mid)
                ctx = holder[0] if holder else None
    _split_waits(nc)
    return nc


def _prep_image(nc, b, env):
    io, pre, col, rep, mm, mp, ps, psd, dr = (env[k] for k in
        ("io", "pre", "col", "rep", "mm", "mp", "ps", "psd", "dr"))
    ident = env["ident"]
    ident16 = env["ident16"]
    d2b = env["d2b"]
    pb, pl, rol, rsl, rob, rsb, rv, tsz = (env[k] for k in
        ("pb", "pl", "rol", "rsl", "rob", "rsb", "rv", "tsz"))
    out_s, out_o = env["out_s"], env["out_o"]

    # image-level scalars: W, H broadcast to all partitions (px and /256)
    WH = col.tile([128, 2], F32, tag="WH")
    nc.sync.dma_start(out=WH, in_=_bcast(tsz[b:b + 1, :], 128))
    Ht = WH[:, 0:1]
    Wt = WH[:, 1:2]
    HtP = WH[:P, 0:1]
    WtP = WH[:P, 1:2]
    Ws = col.tile([128, 1], F32, tag="Ws")
    Hs = col.tile([128, 1], F32, tag="Hs")
    nWs = col.tile([128, 1], F32, tag="nWs")
    nHs = col.tile([128, 1], F32, tag="nHs")
    nc.vector.tensor_scalar(out=Ws, in0=Wt, scalar1=SCALE, scalar2=None,
                            op0=OP.mult)
    nc.vector.tensor_scalar(out=Hs, in0=Ht, scalar1=SCALE, scalar2=None,
                            op0=OP.mult)
    nc.vector.tensor_scalar(out=nWs, in0=Wt, scalar1=-SCALE, scalar2=None,
                            op0=OP.mult)
    nc.vector.tensor_scalar(out=nHs, in0=Ht, scalar1=-SCALE, scalar2=None,
                            op0=OP.mult)

    # --- batched box prep: ent / rs / ro processed in one [125,12,*] pass ---
    # columns of PIXA: 0 x0s 1 y0s 2 nx1s 3 ny1s 4 ws 5 hs 6 areas
    BTA = io.tile([P, 3, NCH, 4], F32, tag="BTA")
    for t, dram in enumerate((pb, rsb, rob)):
        nc.sync.dma_start(out=BTA[:, t, :, :],
                          in_=dram[b].rearrange("(j p) c -> p j c", p=P))
    BTF = BTA[:, :, :, :].rearrange("p t j c -> p (t j) c")
    LO = col.tile([P, 12, 2], F32, tag="LO")
    HIc = col.tile([P, 12, 2], F32, tag="HIc")
    nc.vector.scalar_tensor_tensor(
        out=LO, in0=BTF[:, :, 2:4], scalar=-0.5, in1=BTF[:, :, 0:2],
        op0=OP.mult, op1=OP.add)
    nc.vector.scalar_tensor_tensor(
        out=HIc, in0=BTF[:, :, 2:4], scalar=0.5, in1=BTF[:, :, 0:2],
        op0=OP.mult, op1=OP.add)
    PIXA = col.tile([P, 3, NCH, 7], F32, tag="PIXA")
    PIXF = PIXA[:, :, :, :].rearrange("p t j c -> p (t j) c")
    nc.vector.tensor_scalar(out=PIXF[:, :, 0:1], in0=LO[:, :, 0:1],
                            scalar1=Ws[:P], scalar2=None, op0=OP.mult)
    nc.vector.tensor_scalar(out=PIXF[:, :, 1:2], in0=LO[:, :, 1:2],
                            scalar1=Hs[:P], scalar2=None, op0=OP.mult)
    nc.vector.tensor_scalar(out=PIXF[:, :, 2:3], in0=HIc[:, :, 0:1],
                            scalar1=nWs[:P], scalar2=None, op0=OP.mult)
    nc.vector.tensor_scalar(out=PIXF[:, :, 3:4], in0=HIc[:, :, 1:2],
                            scalar1=nHs[:P], scalar2=None, op0=OP.mult)
    nc.vector.tensor_scalar(out=PIXF[:, :, 4:5], in0=BTF[:, :, 2:3],
                            scalar1=Ws[:P], scalar2=None, op0=OP.mult)
    nc.vector.tensor_scalar(out=PIXF[:, :, 5:6], in0=BTF[:, :, 3:4],
                            scalar1=Hs[:P], scalar2=None, op0=OP.mult)
    nc.vector.tensor_tensor(out=PIXF[:, :, 6:7], in0=PIXF[:, :, 4:5],
                            in1=PIXF[:, :, 5:6], op=OP.mult)
    PIX_E = PIXA[:, 0, :, :]
    PIX_S = PIXA[:, 1, :, :]
    PIX_O = PIXA[:, 2, :, :]

    # PG columns (f32): 0 ws 1 hs 2 areas 3 score | 4 x0s 5 y0s 6 nx1s 7 ny1s
    #                   8 cx_px 9 cy_px   (fp16-bound rows first: partition-0
    #                   aligned reads after the PE transpose)
    PG = pre.tile([P, NCH, 10], F32, tag="PG")
    nc.vector.tensor_copy(out=PG[:, :, 0:3], in_=PIX_E[:, :, 4:7])
    nc.vector.tensor_copy(out=PG[:, :, 4:8], in_=PIX_E[:, :, 0:4])
    nc.vector.tensor_scalar(out=PG[:, :, 8:9], in0=BTA[:, 0, :, 0:1],
                            scalar1=WtP, scalar2=None, op0=OP.mult)
    nc.vector.tensor_scalar(out=PG[:, :, 9:10], in0=BTA[:, 0, :, 1:2],
                            scalar1=HtP, scalar2=None, op0=OP.mult)

    # rel_vec endpoints, negated, px units (ACT bias for |cx - vx|).
    # WH4 = [w,h,w,h] per partition via a reversed-stride broadcast DMA of
    # target_sizes ([h,w] in DRAM); VN = (-rel_vec) * WH4 in one op.
    RVt = io.tile([P, NCH, 4], F32, tag="RVt")
    nc.sync.dma_start(out=RVt, in_=rv[b].rearrange("(j p) c -> p j c", p=P))
    WH4 = col.tile([128, 4], F32, tag="WH4")
    _wh = WH[:, :]
    nc.vector.tensor_copy(out=WH4, in_=bass.AP(
        tensor=_wh.tensor, offset=_wh.offset + 1,
        ap=[list(_wh.ap[0]), [0, 2], [-1, 2]]))
    VN = col.tile([P, NCH, 4], F32, tag="VN")
    _w4 = WH4[:P]
    nc.vector.scalar_tensor_tensor(
        out=VN, in0=RVt, scalar=-1.0,
        in1=bass.AP(tensor=_w4.tensor, offset=_w4.offset,
                    ap=[list(_w4.ap[0]), [0, NCH]] + list(_w4.ap[1:])),
        op0=OP.mult, op1=OP.mult)

    # softmax + fp16 packs + PE transposes into class-major matmul operands
    RHS_A = mm.tile([128, NE], F16, tag="RHS_A")
    RHS_B = mm.tile([24, NE], F16, tag="RHS_B")
    LS_A = mm.tile([128, NR], F16, tag="LS_A")
    LS_B = mm.tile([24, NR], F16, tag="LS_B")
    LO_A = mm.tile([128, NR], F16, tag="LO_A")
    LO_B = mm.tile([24, NR], F16, tag="LO_B")

    for t, (ldram, dA, dB) in {
        "ent": (pl, RHS_A, RHS_B),
        "rs": (rsl, LS_A, LS_B),
        "ro": (rol, LO_A, LO_B),
    }.items():
        LT = io.tile([P, NCH, NC1], F32, tag="LT")
        nc.sync.dma_start(out=LT, in_=ldram[b].rearrange("(j p) c -> p j c",
                                                         p=P))
        # per-chunk exp/square accumulators land in [P, NCH] column tiles so
        # the tiny scalar algebra runs once per image, not once per chunk
        E4 = pre.tile([P, NCH, NC1], F16, tag="E4")
        SM = col.tile([P, NCH, 1], F32, tag="SM")
        SQC = col.tile([P, NCH, 1], F32, tag="SQC")
        for j in range(NCH):
            nc.scalar.activation(out=E4[:, j, :], in_=LT[:, j, :],
                                 func=AF.Exp, accum_out=SM[:, j, :])
            SQ = pre.tile([P, NCL], F16, tag="SQ")
            nc.scalar.activation(out=SQ, in_=E4[:, j, :NCL], func=AF.Square,
                                 accum_out=SQC[:, j, :])
        R4 = col.tile([P, NCH, 1], F32, tag="R4")
        nc.vector.reciprocal(R4, SM)
        pk_s2 = None if t == "ent" else -2.0
        PKs = []
        for j in range(NCH):
            PK = pre.tile([P, 152], F16, tag="PK")
            PKs.append(PK)
            if pk_s2 is None:
                nc.vector.tensor_scalar(out=PK[:, 0:NCL], in0=E4[:, j, :NCL],
                                        scalar1=R4[:, j, :], scalar2=None,
                                        op0=OP.mult)
            else:
                nc.vector.tensor_scalar(out=PK[:, 0:NCL], in0=E4[:, j, :NCL],
                                        scalar1=R4[:, j, :], scalar2=pk_s2,
                                        op0=OP.mult, op1=OP.mult)
        r2 = col.tile([P, NCH, 1], F32, tag="r2")
        nc.vector.tensor_tensor(out=r2, in0=R4, in1=R4, op=OP.mult)
        if t == "ent":
            MX = col.tile([P, NCH, 1], F32, tag="MX")
            nc.vector.tensor_reduce(out=MX, in_=E4[:, :, :NCL],
                                    axis=mybir.AxisListType.X, op=OP.max)
            PK151 = col.tile([P, NCH, 1], F32, tag="PK151")
            nc.vector.tensor_tensor(out=PK151, in0=SQC, in1=r2, op=OP.mult)
            nc.vector.tensor_tensor(out=PG[:, :, 3:4], in0=MX, in1=R4,
                                    op=OP.mult)
            k150, k151 = 0.25, PK151
        else:
            PK150 = col.tile([P, NCH, 1], F32, tag="PK150")
            nc.vector.scalar_tensor_tensor(out=PK150, in0=r2, scalar=4.0,
                                           in1=SQC, op0=OP.mult, op1=OP.mult)
            k150, k151 = PK150, 1.0
        for j in range(NCH):
            PK = PKs[j]
            if t == "ent":
                nc.vector.memset(PK[:, 150:151], k150)
                nc.vector.tensor_copy(out=PK[:, 151:152], in_=k151[:, j, :])
            else:
                nc.vector.tensor_copy(out=PK[:, 150:151], in_=k150[:, j, :])
                nc.vector.memset(PK[:, 151:152], k151)
            TA = ps.tile([128, P], F16, tag="TT16")
            nc.tensor.transpose(TA, PK[:, 0:128], ident16[:P, :P])
            nc.scalar.copy(out=dA[:, P * j:P * (j + 1)], in_=TA)
            TB = ps.tile([24, P], F16, tag="TT16")
            nc.tensor.transpose(TB, PK[:, 128:152], ident16[:P, :P])
            nc.scalar.copy(out=dB[:, P * j:P * (j + 1)], in_=TB)

    # entity-side rows -> DRAM -> broadcast into replicated tiles.
    # EROWS rows: 0 x0s 1 y0s 2 nx1s 3 ny1s 4 cx 5 cy | 6 ws 7 hs 8 areas 9 scr
    # rows 0..5 stay f32 (one [P,6,NE] broadcast); rows 6..9 are converted to
    # fp16 and broadcast DOUBLED ([P,4,2,NE]) for the fused sub/obj map ops.
    EROWS = rep.tile([10, NE], F32, tag="EROWS")
    for j in range(NCH):
        TE = ps.tile([10, P], F32, tag="TT")
        nc.tensor.transpose(TE, PG[:, j, :], ident[:P, :P])
        nc.scalar.copy(out=EROWS[:, P * j:P * (j + 1)], in_=TE)
    EROWS16 = rep.tile([4, NE], F16, tag="EROWS16")
    nc.vector.tensor_copy(out=EROWS16, in_=EROWS[0:4, :])
    ED32 = dr.tile([6, NE], F32, tag="ED32", name="ED32")
    nc.sync.dma_start(out=ED32, in_=EROWS[4:10, :])
    ED16 = dr.tile([4, NE], F16, tag="ED16", name="ED16")
    nc.sync.dma_start(out=ED16, in_=EROWS16)
    RALL = rep.tile([P, 6, NE], F32, tag="RALL", name="RALL")
    _ed = ED32[:, :]
    nc.sync.dma_start(out=RALL, in_=bass.AP(
        tensor=_ed.tensor, offset=_ed.offset,
        ap=[[0, P]] + list(_ed.ap)))
    RD4 = rep.tile([P, 4, 2, NE], F16, tag="RD4", name="RD4")
    _eh = ED16[:, :]
    for k in range(2):
        nc.sync.dma_start(out=RD4[:, :, k, :], in_=bass.AP(
            tensor=_eh.tensor, offset=_eh.offset,
            ap=[[0, P]] + list(_eh.ap)))
    R = {}
    for k, nm_ in enumerate(["X0R", "Y0R", "X1N", "Y1N", "CXR", "CYR"]):
        R[nm_] = RALL[:, k, :]
    for k, nm_ in enumerate(["WER2", "HER2", "AREAR2", "SCR2"]):
        R[nm_] = RD4[:, k, :, :]
    return dict(R=R, VN=VN, PIX_S=PIX_S, PIX_O=PIX_O, PIXA=PIXA, RALL=RALL,
                RD4=RD4,
                RHS_A=RHS_A, RHS_B=RHS_B, LS_A=LS_A, LS_B=LS_B,
                LO_A=LO_A, LO_B=LO_B)


def _maps_image(nc, b, env, ctx, mid_thunk):
    io, pre, col, rep, mm, mp, ps, psd, dr = (env[k] for k in
        ("io", "pre", "col", "rep", "mm", "mp", "ps", "psd", "dr"))
    d2b = env["d2b"]
    out_s, out_o = env["out_s"], env["out_o"]
    R = ctx["R"]
    VN = ctx["VN"]
    PIXA = ctx["PIXA"]
    RALL = ctx["RALL"]
    RD4 = ctx["RD4"]
    RHS_A, RHS_B = ctx["RHS_A"], ctx["RHS_B"]

    # fused sub/obj map pipeline: per chunk j, both maps are computed in
    # [P, 2, NE] pair tiles; ops without map-specific scalars run once over
    # the doubled free dim.
    MAPS = ((ctx["LS_A"], ctx["LS_B"], ctx["PIX_S"], 0, 1, out_s),
            (ctx["LO_A"], ctx["LO_B"], ctx["PIX_O"], 2, 3, out_o))

    def m2(tag, dt=F16):
        return mp.tile([P, 2, NE], dt, tag=tag, name=tag)

    def eng(name):
        return nc.gpsimd if name in POOL_OPS else nc.vector

    def stageA_make(j):
        """Allocate pair-j stage-A tiles; return (tiles, dve_thunks,
        pe_thunks, act_thunks). DVE thunks are ordered so no op reads a tile
        written by the immediately-preceding DVE op (write-to-read stall)."""
        ax_p, ay_p, s_p = m2("ax"), m2("ay"), m2("s")
        dxn = m2("dxn")
        dyn = m2("dyn")
        vx1n = [mp.tile([P, NE], F32, tag="vx1n", name="vx1n")
                for _ in range(2)]
        vy1n = [mp.tile([P, NE], F32, tag="vy1n", name="vy1n")
                for _ in range(2)]
        D2s = [psd.tile([P, NE], F32, tag="D2", name="D2") for _ in range(2)]
        pe, act, dve = [], [], []
        for mi, (lA, lB, PIXR, vxc, vyc, odram) in enumerate(MAPS):
            def _mm(mi=mi, lA=lA, lB=lB):
                nc.tensor.matmul(D2s[mi], lhsT=lA[:, P * j:P * (j + 1)],
                                 rhs=RHS_A, start=True, stop=False)
                nc.tensor.matmul(D2s[mi], lhsT=lB[:, P * j:P * (j + 1)],
                                 rhs=RHS_B, start=False, stop=True)
            pe.append(_mm)
            act.append(lambda mi=mi, vxc=vxc: nc.scalar.activation(
                out=ax_p[:, mi, :], in_=R["CXR"], func=AF.Abs,
                bias=VN[:, j, vxc:vxc + 1]))
            act.append(lambda mi=mi, vyc=vyc: nc.scalar.activation(
                out=ay_p[:, mi, :], in_=R["CYR"], func=AF.Abs,
                bias=VN[:, j, vyc:vyc + 1]))
            dve.append(lambda mi=mi, PIXR=PIXR: nc.vector.tensor_scalar(
                out=vx1n[mi], in0=R["X1N"], scalar1=PIXR[:, j, 2:3],
                scalar2=None, op0=OP.max))
            dve.append(lambda mi=mi, PIXR=PIXR: nc.vector.tensor_scalar(
                out=vy1n[mi], in0=R["Y1N"], scalar1=PIXR[:, j, 3:4],
                scalar2=None, op0=OP.max))
        for mi, (lA, lB, PIXR, vxc, vyc, odram) in enumerate(MAPS):
            dve.append(lambda mi=mi, PIXR=PIXR: nc.vector.scalar_tensor_tensor(
                out=dxn[:, mi, :], in0=R["X0R"], scalar=PIXR[:, j, 0:1],
                in1=vx1n[mi], op0=OP.max, op1=OP.add))
            dve.append(lambda mi=mi, PIXR=PIXR: nc.vector.scalar_tensor_tensor(
                out=dyn[:, mi, :], in0=R["Y0R"], scalar=PIXR[:, j, 1:2],
                in1=vy1n[mi], op0=OP.max, op1=OP.add))
        for mi in range(2):
            act.append(lambda mi=mi: nc.scalar.activation(
                out=s_p[:, mi, :], in_=D2s[mi], func=AF.Sqrt, bias=d2b[:P]))
        return (ax_p, ay_p, s_p, dxn, dyn), dve, pe, act

    def fusedStage(j, stA, filler, tail_thunks):
        ax_p, ay_p, s_p, dxn, dyn = stA

        def F():
            for th in filler:
                th()
                return

        def ts(name, in0, s1, op0, s2=None, op1=None, dt=F16, out=None):
            o = out if out is not None else m2(name, dt)
            tgt = o if out is None else out
            if op1 is None:
                eng(name).tensor_scalar(out=tgt, in0=in0, scalar1=s1,
                                        scalar2=None, op0=op0)
            else:
                eng(name).tensor_scalar(out=tgt, in0=in0, scalar1=s1,
                                        scalar2=s2, op0=op0, op1=op1)
            return o

        def tt(name, in0, in1, op, dt=F16):
            o = m2(name, dt)
            eng(name).tensor_tensor(out=o, in0=in0, in1=in1, op=op)
            return o

        PIX0 = MAPS[0][2]
        PIX1 = MAPS[1][2]
        # ---- fused stage; F() = one stage-A(j+1) DVE op as a gap spacer ----
        i1 = m2("i1")
        nc.vector.scalar_tensor_tensor(                        # relu(dx)*dy
            out=i1, in0=dxn, scalar=0.0, in1=dyn, op0=OP.min, op1=OP.mult)
        d12 = tt("d12", ax_p, ay_p, OP.add)                    # pool
        F()
        I2 = m2("I2")
        w1 = m2("w1")
        h1 = m2("h1")
        ts("I2", i1[:, 0, :], 0.0, OP.max, PIX0[:, j, 6:7], OP.subtract,
           out=I2[:, 0, :])
        ts("w1", dxn[:, 0, :], PIX0[:, j, 4:5], OP.add, out=w1[:, 0, :])
        F()
        ts("I2", i1[:, 1, :], 0.0, OP.max, PIX1[:, j, 6:7], OP.subtract,
           out=I2[:, 1, :])
        ts("w1", dxn[:, 1, :], PIX1[:, j, 4:5], OP.add, out=w1[:, 1, :])
        F()
        U = tt("U", R["AREAR2"], I2, OP.subtract)              # union
        ts("h1", dyn[:, 0, :], PIX0[:, j, 5:6], OP.add, out=h1[:, 0, :])
        ts("h1", dyn[:, 1, :], PIX1[:, j, 5:6], OP.add, out=h1[:, 1, :])
        F()
        mI = m2("mI")
        nc.vector.scalar_tensor_tensor(                        # I - u
            out=mI, in0=i1, scalar=0.0, in1=U, op0=OP.max, op1=OP.subtract)
        sq = m2("sq")
        nc.scalar.activation(out=sq, in_=U, func=AF.Square)    # u^2
        wc = tt("wc", w1, R["WER2"], OP.add)
        F()
        hc = tt("hc", h1, R["HER2"], OP.add)
        F()
        A = tt("A", wc, hc, OP.mult)                           # areac
        s1 = ts("s1", s_p, 1.0, OP.add)
        F()
        prod = tt("prod", A, mI, OP.mult)
        Pn = tt("Pn", U, A, OP.mult)                           # pool
        ds1 = ts("ds1", d12, 1.0, OP.add)
        F()
        N = tt("N", sq, prod, OP.add)                          # numerator
        den = tt("den", s1, ds1, OP.mult)                      # pool
        D3 = tt("D3", Pn, den, OP.mult, dt=F32)                # pool, > 0
        # 1/D3 on ScalarE via exp(-ln(D3)) - keeps the (slow, iterative)
        # DVE reciprocal off the bottleneck engine.
        lg = m2("lg")
        nc.scalar.activation(out=lg, in_=D3, func=AF.Ln)
        r3 = m2("r3", F32)
        nc.scalar.activation(out=r3, in_=lg, func=AF.Exp, scale=-1.0)
        F()
        nm = m2("nm")
        nc.vector.scalar_tensor_tensor(out=nm, in0=N, scalar=0.0,
                                       in1=r3, op0=OP.max, op1=OP.mult)
        for th in tail_thunks:
            th()
        outv = tt("outv", nm, R["SCR2"], OP.mult)              # pool
        for mi, (lA, lB, PIXR, vxc, vyc, odram) in enumerate(MAPS):
            nc.sync.dma_start(out=odram[b, P * j:P * (j + 1), :],
                              in_=outv[:, mi, :])
        for th in filler:
            th()

    # depth-2 software pipeline with fine-grained interleave: pair j+1's
    # independent DVE ops are woven between pair j's dependent ops so the
    # DVE never reads a tile written by its immediately-preceding op.
    stA, dveA, peA, actA = stageA_make(0)
    for th in peA + actA + dveA:
        th()
    for j in range(NCH):
        if j + 1 < NCH:
            nxt, dveN, peN, actN = stageA_make(j + 1)
            for th in peN:
                th()
        else:
            nxt, dveN, actN = None, [], []
        fusedStage(j, stA, iter(dveN), actN)
        stA = nxt
        if j == 0:
            mid_thunk()  # next image prep hides in this image's map slack


class _CompiledKernel:
    """Compiled SPMD executable: jit built once, reusable across calls."""

    def __init__(self, nc, n_cores):
        import jax
        from jax.sharding import Mesh, PartitionSpec
        try:
            from jax.experimental.shard_map import shard_map
        except Exception:
            from jax.shard_map import shard_map
        from concourse import bass2jax
        from concourse.bass2jax import _bass_exec_p, install_neuronx_cc_hook

        install_neuronx_cc_hook()
        self.jax = jax
        self.n_cores = n_cores
        partition_name = (nc.partition_id_tensor.name
                          if nc.partition_id_tensor else None)
        in_names, out_names, out_avals, zero_outs = [], [], [], []
        for alloc in nc.m.functions[0].allocations:
            if not isinstance(alloc, mybir.MemoryLocationSet):
                continue
            name = alloc.memorylocations[0].name
            if alloc.kind == "ExternalInput":
                if name != partition_name:
                    in_names.append(name)
            elif alloc.kind == "ExternalOutput":
                shape = tuple(alloc.tensor_shape)
                dtype = mybir.dt.np(alloc.dtype)
                out_names.append(name)
                out_avals.append(jax.core.ShapedArray(shape, dtype))
                zero_outs.append(np.zeros(shape, dtype))
        self.in_names = in_names
        self.out_names = out_names
        self.out_avals = out_avals
        self.zero_outs = zero_outs
        all_in = in_names + out_names
        if partition_name is not None:
            all_in.append(partition_name)

        def _body(*args):
            operands = list(args)
            if partition_name is not None:
                operands.append(bass2jax.partition_id_tensor())
            return tuple(_bass_exec_p.bind(
                *operands,
                out_avals=tuple(out_avals),
                in_names=tuple(all_in),
                out_names=tuple(out_names),
                lowering_input_output_aliases=(),
                sim_require_finite=True,
                sim_require_nnan=True,
                nc=nc,
            ))

        devices = jax.devices()[:n_cores]
        self._mesh = Mesh(np.asarray(devices), ("core",))
        nin = len(in_names) + len(out_names)
        sm = shard_map(_body, mesh=self._mesh,
                       in_specs=(PartitionSpec("core"),) * nin,
                       out_specs=(PartitionSpec("core"),) * len(out_names),
                       check_rep=False)
        from jax.sharding import NamedSharding
        sh = NamedSharding(self._mesh, PartitionSpec("core"))
        in_abst = []
        for alloc in nc.m.functions[0].allocations:
            if not isinstance(alloc, mybir.MemoryLocationSet):
                continue
            name = alloc.memorylocations[0].name
            if alloc.kind == "ExternalInput" and name in in_names:
                shape = tuple(alloc.tensor_shape)
                in_abst.append(jax.ShapeDtypeStruct(
                    (n_cores * shape[0], *shape[1:]), mybir.dt.np(alloc.dtype),
                    sharding=sh))
        out_abst = [jax.ShapeDtypeStruct((n_cores * z.shape[0], *z.shape[1:]),
                                         z.dtype, sharding=sh)
                    for z in self.zero_outs]
        try:
            from concourse.bass2jax import fast_dispatch_compile
            self._fn = fast_dispatch_compile(
                lambda: jax.jit(sm, keep_unused=True)
                .lower(*in_abst, *out_abst).compile())
        except Exception:
            self._fn = jax.jit(sm, keep_unused=True)

    def run(self, in_maps):
        jax = self.jax
        n = self.n_cores
        per_core = [[np.asarray(m[nm]) for nm in self.in_names]
                    for m in in_maps]
        concat_in = [np.concatenate([per_core[c][i] for c in range(n)], axis=0)
                     for i in range(len(self.in_names))]
        concat_zero = [np.zeros((n * z.shape[0], *z.shape[1:]), z.dtype)
                       for z in self.zero_outs]
        outs = jax.block_until_ready(self._fn(*concat_in, *concat_zero))
        return [
            {nm: np.asarray(outs[i]).reshape(n, *self.out_avals[i].shape)[c]
             for i, nm in enumerate(self.out_names)}
            for c in range(n)
        ]


_CACHE = {}


def _get_nc(reps=1):
    key = ("nc", reps)
    if key not in _CACHE:
        _CACHE[key] = _build(N_IMG, reps=reps)
    return _CACHE[key]


def _get_ck(reps=1):
    key = ("ck", reps)
    if key not in _CACHE:
        _CACHE[key] = _CompiledKernel(_get_nc(reps), N_CORES)
    return _CACHE[key]


def kernel(pred_boxes, pred_logits, pred_rel_obj_logits, pred_rel_sub_logits,
           pred_rel_obj_box, pred_rel_sub_box, pred_rel_vec, target_sizes):
    inp = {
        "pred_boxes": np.ascontiguousarray(pred_boxes, dtype=np.float32),
        "pred_logits": np.ascontiguousarray(pred_logits, dtype=np.float32),
        "pred_rel_obj_logits": np.ascontiguousarray(pred_rel_obj_logits, dtype=np.float32),
        "pred_rel_sub_logits": np.ascontiguousarray(pred_rel_sub_logits, dtype=np.float32),
        "pred_rel_obj_box": np.ascontiguousarray(pred_rel_obj_box, dtype=np.float32),
        "pred_rel_sub_box": np.ascontiguousarray(pred_rel_sub_box, dtype=np.float32),
        "pred_rel_vec": np.ascontiguousarray(pred_rel_vec, dtype=np.float32),
        "target_sizes": np.ascontiguousarray(target_sizes, dtype=np.float32),
    }
    in_maps = [{k: v[c * N_IMG:(c + 1) * N_IMG] for k, v in inp.items()}
               for c in range(N_CORES)]
    res = None
    try:
        res = _get_ck().run(in_maps)
    except Exception:
        import time as _time
        _time.sleep(2.0)
        try:
            res = _get_ck().run(in_maps)
        except Exception:
            r = bass_utils.run_bass_kernel_spmd(
                _get_nc(), in_maps, core_ids=list(range(N_CORES)))
            res = r.results
    sub = np.concatenate([res[c]["out_sub"] for c in range(N_CORES)], axis=0)
    obj = np.concatenate([res[c]["out_obj"] for c in range(N_CORES)], axis=0)
    return np.float32(sub), np.float32(obj)



# revision 5
# speedup vs baseline: 7.1876x; 1.0027x over previous
"""Trainium2 Bass kernel for nn_EntitiesIndexingHeadRuleBased (nms_detection).

kernel(**inputs) takes the FULL batch (B=64) and returns (sub_dist, obj_dist),
each [64, 500, 500] float32, matching the reference semantics:

  out_s[r,e] = relu(N) * score_e / (u*A*(d+1)*(s+1))
  N          = u^2 - A*(u - I)        (algebraic form of clip(giou,0)*u*A)
  u, A, I    = union, enclosing area, intersection (ent_e box vs rel box)
  d          = |vx-cx_e| + |vy-cy_e| ;  s = sqrt(cdist^2 via matmul)

Sharding: pure data parallelism - batch 64 split as 8 images per NeuronCore
across 8 cores (SPMD, one Bass program).

Performance notes (v2):
  * fp16 intermediates: tensor_scalar runs in 4x DVE mode, tensor_tensor in
    2x; scalar_tensor_tensor (1x always) is reserved for the few
    cancellation-sensitive ops which compute in the fp32 ALU and emit
    value-scale fp16 (keeps rel-err ~5e-4 despite fp16 storage).
  * box coords are pre-scaled by 1/256 so all giou quantities fit fp16 range.
  * matmul packs are fp16 (1 cyc/row on PE instead of 4 for fp32).
  * a slice of the per-chunk map ops runs on GPSIMD (Pool) to unload DVE.
  * output is fp16 (halves write traffic); host converts to f32.
"""
import sys
sys.path.insert(0, '/opt/trn_rl_repo')

import numpy as np
import bass_rust
import concourse.bass as bass
import concourse.tile as tile
import concourse.tile as tile_mod
from concourse import mybir
from concourse import bass_utils
from concourse.masks import make_identity
from concourse.tile import TileContext

F32 = mybir.dt.float32
F16 = mybir.dt.float16
AF = mybir.ActivationFunctionType
OP = mybir.AluOpType

B = 64
NE = 500
NR = 500
NC1 = 151
NCL = 150
P = 125
NCH = 4
N_CORES = 8
N_IMG = B // N_CORES

SCALE = 1.0 / 256.0          # box-coordinate prescale for fp16 range
D2_BIAS = 3e-4               # clamp for sqrt(d2) against fp16 rounding

# Which map ops run on GPSIMD (Pool) instead of DVE. Walrus only accepts
# plain TensorTensor (add/mult) there - TensorScalarPtr fails engine check.
POOL_OPS = frozenset({"d12", "outv", "Pn", "den", "D3"})

# ---------------------------------------------------------------------------
# Workarounds for the container's walrus: it rejects instructions carrying
# more than one sync-wait command ("Too many sync wait commands").
# ---------------------------------------------------------------------------

_MAXW = 1


def _patched_drain_and_barrier(self, tick_clock, wait_clock):
    ScopedClock = tile_mod.ScopedClock
    carrier = self.nc.sync.nop(nofuse=True)
    wait_clock.add_sem_waits(carrier.ins,
                             ScopedClock({None: tick_clock.global_clock}))
    si = carrier.ins.sync_info
    waits = list(si.on_wait) if si is not None else []
    if len(waits) > _MAXW:
        carrier.ins.sync_info = bass_rust.SyncInfo(
            on_wait=waits[:_MAXW], on_update=[])
        for i in range(_MAXW, len(waits), _MAXW):
            nop = self.nc.sync.nop(nofuse=True)
            nop.ins.sync_info = bass_rust.SyncInfo(
                on_wait=waits[i:i + _MAXW], on_update=[])
    self.nc.sync.drain()
    self.nc.all_engine_barrier()
    assert self.sems is not None
    popped = self.nc._tile_sem_poison_stack.pop()
    assert popped is self._sem_poison
    self.nc.clear_and_free_semaphores(list(self.sems.allocated().values()))
    self.nc.all_engine_barrier()


TileContext._drain_and_barrier = _patched_drain_and_barrier


def _split_waits(nc, maxw=_MAXW):
    """Hoist excess sync waits onto same-engine NoOps placed just before the
    offending instruction (engine streams execute in order)."""
    for fn in nc.m.functions:
        for blk in fn.blocks:
            newl = []
            changed = False
            for ins in blk.instructions:
                si = ins.sync_info
                waits = list(si.on_wait) if si is not None else []
                if len(waits) > maxw:
                    changed = True
                    carried, rest = waits[:-maxw], waits[-maxw:]
                    for i in range(0, len(carried), maxw):
                        nop = mybir.InstNoOp(
                            name=f"{ins.name}-sw{i}",
                            sync_info=mybir.SyncInfo(
                                on_wait=carried[i:i + maxw], on_update=[]),
                            bass_nofuse=True,
                            engine=ins.engine,
                        )
                        newl.append(nop)
                    ins.sync_info = mybir.SyncInfo(
                        on_wait=rest, on_update=list(si.on_update))
                newl.append(ins)
            if changed:
                blk.instructions = newl


# ---------------------------------------------------------------------------
# Kernel builder
# ---------------------------------------------------------------------------

def _bcast(ap, p):
    """[1,N] DRAM AP -> [p,N] partition-broadcast AP (stride-0 partition)."""
    return bass.AP(tensor=ap.tensor, offset=ap.offset,
                   ap=[[0, p]] + list(ap.ap[1:]))


def _build(n_img, reps=1):
    nc = bass.Bass("TRN2", target_bir_lowering=False)

    pb = nc.dram_tensor("pred_boxes", [n_img, NE, 4], F32, kind="ExternalInput")
    pl = nc.dram_tensor("pred_logits", [n_img, NE, NC1], F32, kind="ExternalInput")
    rol = nc.dram_tensor("pred_rel_obj_logits", [n_img, NR, NC1], F32, kind="ExternalInput")
    rsl = nc.dram_tensor("pred_rel_sub_logits", [n_img, NR, NC1], F32, kind="ExternalInput")
    rob = nc.dram_tensor("pred_rel_obj_box", [n_img, NR, 4], F32, kind="ExternalInput")
    rsb = nc.dram_tensor("pred_rel_sub_box", [n_img, NR, 4], F32, kind="ExternalInput")
    rv = nc.dram_tensor("pred_rel_vec", [n_img, NR, 4], F32, kind="ExternalInput")
    tsz = nc.dram_tensor("target_sizes", [n_img, 2], F32, kind="ExternalInput")
    out_s = nc.dram_tensor("out_sub", [n_img, NR, NE], F16, kind="ExternalOutput")
    out_o = nc.dram_tensor("out_obj", [n_img, NR, NE], F16, kind="ExternalOutput")

    with tile.TileContext(nc) as tc:
        with (
            tc.tile_pool(name="singles", bufs=1) as singles,
            tc.tile_pool(name="io", bufs=3) as io,
            tc.tile_pool(name="pre", bufs=5) as pre,
            tc.tile_pool(name="col", bufs=2) as col,
            tc.tile_pool(name="rep", bufs=2) as rep,
            tc.tile_pool(name="mm", bufs=2) as mm,
            tc.tile_pool(name="mp", bufs=2) as mp,
            tc.tile_pool(name="ps", bufs=2, space="PSUM") as ps,
            tc.tile_pool(name="psd", bufs=4, space="PSUM") as psd,
            tc.tile_pool(name="dr", bufs=2, space="DRAM") as dr,
        ):
            ident = singles.tile([128, 128], F32, tag="ident")
            make_identity(nc, ident)
            ident16 = singles.tile([128, 128], F16, tag="ident16")
            nc.vector.tensor_copy(out=ident16, in_=ident)
            d2b = singles.tile([128, 1], F32, tag="d2b")
            nc.vector.memset(d2b, D2_BIAS)

            # image-level software pipeline: image b+1's prep (softmax,
            # boxes, broadcasts) is emitted in the middle of image b's map
            # stage so its ACT/DVE/DMA work hides in the map stage's slack.
            # reps>1 repeats the whole batch back-to-back (timing variant);
            # the pipeline runs straight through the seam.
            env = locals()
            seq = [i % n_img for i in range(n_img * reps)]
            ctx = _prep_image(nc, seq[0], env)
            for k, b in enumerate(seq):
                holder = []
                if k + 1 < len(seq):
                    def mid(bn=seq[k + 1], holder=holder):
                        holder.append(_prep_image(nc, bn, env))
                else:
                    def mid():
                        return None
                _maps_image(nc, b, env, ctx, mid)
                ctx = holder[0] if holder else None
    _split_waits(nc)
    return nc


def _prep_image(nc, b, env):
    io, pre, col, rep, mm, mp, ps, psd, dr = (env[k] for k in
        ("io", "pre", "col", "rep", "mm", "mp", "ps", "psd", "dr"))
    ident = env["ident"]
    ident16 = env["ident16"]
    d2b = env["d2b"]
    pb, pl, rol, rsl, rob, rsb, rv, tsz = (env[k] for k in
        ("pb", "pl", "rol", "rsl", "rob", "rsb", "rv", "tsz"))
    out_s, out_o = env["out_s"], env["out_o"]

    # image-level scalars: W, H broadcast to all partitions (px and /256)
    WH = col.tile([128, 2], F32, tag="WH")
    nc.sync.dma_start(out=WH, in_=_bcast(tsz[b:b + 1, :], 128))
    Ht = WH[:, 0:1]
    Wt = WH[:, 1:2]
    HtP = WH[:P, 0:1]
    WtP = WH[:P, 1:2]
    Ws = col.tile([128, 1], F32, tag="Ws")
    Hs = col.tile([128, 1], F32, tag="Hs")
    nWs = col.tile([128, 1], F32, tag="nWs")
    nHs = col.tile([128, 1], F32, tag="nHs")
    nc.vector.tensor_scalar(out=Ws, in0=Wt, scalar1=SCALE, scalar2=None,
                            op0=OP.mult)
    nc.vector.tensor_scalar(out=Hs, in0=Ht, scalar1=SCALE, scalar2=None,
                            op0=OP.mult)
    nc.vector.tensor_scalar(out=nWs, in0=Wt, scalar1=-SCALE, scalar2=None,
                            op0=OP.mult)
    nc.vector.tensor_scalar(out=nHs, in0=Ht, scalar1=-SCALE, scalar2=None,
                            op0=OP.mult)

    # --- batched box prep: ent / rs / ro processed in one [125,12,*] pass ---
    # columns of PIXA: 0 x0s 1 y0s 2 nx1s 3 ny1s 4 ws 5 hs 6 areas
    BTA = io.tile([P, 3, NCH, 4], F32, tag="BTA")
    for t, dram in enumerate((pb, rsb, rob)):
        nc.sync.dma_start(out=BTA[:, t, :, :],
                          in_=dram[b].rearrange("(j p) c -> p j c", p=P))
    BTF = BTA[:, :, :, :].rearrange("p t j c -> p (t j) c")
    LO = col.tile([P, 12, 2], F32, tag="LO")
    HIc = col.tile([P, 12, 2], F32, tag="HIc")
    nc.vector.scalar_tensor_tensor(
        out=LO, in0=BTF[:, :, 2:4], scalar=-0.5, in1=BTF[:, :, 0:2],
        op0=OP.mult, op1=OP.add)
    nc.vector.scalar_tensor_tensor(
        out=HIc, in0=BTF[:, :, 2:4], scalar=0.5, in1=BTF[:, :, 0:2],
        op0=OP.mult, op1=OP.add)
    PIXA = col.tile([P, 3, NCH, 7], F32, tag="PIXA")
    PIXF = PIXA[:, :, :, :].rearrange("p t j c -> p (t j) c")
    nc.vector.tensor_scalar(out=PIXF[:, :, 0:1], in0=LO[:, :, 0:1],
                            scalar1=Ws[:P], scalar2=None, op0=OP.mult)
    nc.vector.tensor_scalar(out=PIXF[:, :, 1:2], in0=LO[:, :, 1:2],
                            scalar1=Hs[:P], scalar2=None, op0=OP.mult)
    nc.vector.tensor_scalar(out=PIXF[:, :, 2:3], in0=HIc[:, :, 0:1],
                            scalar1=nWs[:P], scalar2=None, op0=OP.mult)
    nc.vector.tensor_scalar(out=PIXF[:, :, 3:4], in0=HIc[:, :, 1:2],
                            scalar1=nHs[:P], scalar2=None, op0=OP.mult)
    nc.vector.tensor_scalar(out=PIXF[:, :, 4:5], in0=BTF[:, :, 2:3],
                            scalar1=Ws[:P], scalar2=None, op0=OP.mult)
    nc.vector.tensor_scalar(out=PIXF[:, :, 5:6], in0=BTF[:, :, 3:4],
                            scalar1=Hs[:P], scalar2=None, op0=OP.mult)
    nc.vector.tensor_tensor(out=PIXF[:, :, 6:7], in0=PIXF[:, :, 4:5],
                            in1=PIXF[:, :, 5:6], op=OP.mult)
    PIX_E = PIXA[:, 0, :, :]
    PIX_S = PIXA[:, 1, :, :]
    PIX_O = PIXA[:, 2, :, :]

    # PG columns (f32): 0 ws 1 hs 2 areas 3 score | 4 x0s 5 y0s 6 nx1s 7 ny1s
    #                   8 cx_px 9 cy_px   (fp16-bound rows first: partition-0
    #                   aligned reads after the PE transpose)
    PG = pre.tile([P, NCH, 10], F32, tag="PG")
    nc.vector.tensor_copy(out=PG[:, :, 0:3], in_=PIX_E[:, :, 4:7])
    nc.vector.tensor_copy(out=PG[:, :, 4:8], in_=PIX_E[:, :, 0:4])
    nc.vector.tensor_scalar(out=PG[:, :, 8:9], in0=BTA[:, 0, :, 0:1],
                            scalar1=WtP, scalar2=None, op0=OP.mult)
    nc.vector.tensor_scalar(out=PG[:, :, 9:10], in0=BTA[:, 0, :, 1:2],
                            scalar1=HtP, scalar2=None, op0=OP.mult)

    # rel_vec endpoints, negated, px units (ACT bias for |cx - vx|).
    # WH4 = [w,h,w,h] per partition via a reversed-stride broadcast DMA of
    # target_sizes ([h,w] in DRAM); VN = (-rel_vec) * WH4 in one op.
    RVt = io.tile([P, NCH, 4], F32, tag="RVt")
    nc.sync.dma_start(out=RVt, in_=rv[b].rearrange("(j p) c -> p j c", p=P))
    WH4 = col.tile([128, 4], F32, tag="WH4")
    _wh = WH[:, :]
    nc.vector.tensor_copy(out=WH4, in_=bass.AP(
        tensor=_wh.tensor, offset=_wh.offset + 1,
        ap=[list(_wh.ap[0]), [0, 2], [-1, 2]]))
    VN = col.tile([P, NCH, 4], F32, tag="VN")
    _w4 = WH4[:P]
    nc.vector.scalar_tensor_tensor(
        out=VN, in0=RVt, scalar=-1.0,
        in1=bass.AP(tensor=_w4.tensor, offset=_w4.offset,
                    ap=[list(_w4.ap[0]), [0, NCH]] + list(_w4.ap[1:])),
        op0=OP.mult, op1=OP.mult)

    # softmax + fp16 packs + PE transposes into class-major matmul operands
    RHS_A = mm.tile([128, NE], F16, tag="RHS_A")
    RHS_B = mm.tile([24, NE], F16, tag="RHS_B")
    LS_A = mm.tile([128, NR], F16, tag="LS_A")
    LS_B = mm.tile([24, NR], F16, tag="LS_B")
    LO_A = mm.tile([128, NR], F16, tag="LO_A")
    LO_B = mm.tile([24, NR], F16, tag="LO_B")

    for t, (ldram, dA, dB) in {
        "ent": (pl, RHS_A, RHS_B),
        "rs": (rsl, LS_A, LS_B),
        "ro": (rol, LO_A, LO_B),
    }.items():
        LT = io.tile([P, NCH, NC1], F32, tag="LT")
        nc.sync.dma_start(out=LT, in_=ldram[b].rearrange("(j p) c -> p j c",
                                                         p=P))
        # per-chunk exp/square accumulators land in [P, NCH] column tiles so
        # the tiny scalar algebra runs once per image, not once per chunk
        E4 = pre.tile([P, NCH, NC1], F16, tag="E4")
        SM = col.tile([P, NCH, 1], F32, tag="SM")
        SQC = col.tile([P, NCH, 1], F32, tag="SQC")
        for j in range(NCH):
            nc.scalar.activation(out=E4[:, j, :], in_=LT[:, j, :],
                                 func=AF.Exp, accum_out=SM[:, j, :])
            SQ = pre.tile([P, NCL], F16, tag="SQ")
            nc.scalar.activation(out=SQ, in_=E4[:, j, :NCL], func=AF.Square,
                                 accum_out=SQC[:, j, :])
        R4 = col.tile([P, NCH, 1], F32, tag="R4")
        nc.vector.reciprocal(R4, SM)
        pk_s2 = None if t == "ent" else -2.0
        PKs = []
        for j in range(NCH):
            PK = pre.tile([P, 152], F16, tag="PK")
            PKs.append(PK)
            if pk_s2 is None:
                nc.vector.tensor_scalar(out=PK[:, 0:NCL], in0=E4[:, j, :NCL],
                                        scalar1=R4[:, j, :], scalar2=None,
                                        op0=OP.mult)
            else:
                nc.vector.tensor_scalar(out=PK[:, 0:NCL], in0=E4[:, j, :NCL],
                                        scalar1=R4[:, j, :], scalar2=pk_s2,
                                        op0=OP.mult, op1=OP.mult)
        r2 = col.tile([P, NCH, 1], F32, tag="r2")
        nc.vector.tensor_tensor(out=r2, in0=R4, in1=R4, op=OP.mult)
        if t == "ent":
            MX = col.tile([P, NCH, 1], F32, tag="MX")
            nc.vector.tensor_reduce(out=MX, in_=E4[:, :, :NCL],
                                    axis=mybir.AxisListType.X, op=OP.max)
            PK151 = col.tile([P, NCH, 1], F32, tag="PK151")
            nc.vector.tensor_tensor(out=PK151, in0=SQC, in1=r2, op=OP.mult)
            nc.vector.tensor_tensor(out=PG[:, :, 3:4], in0=MX, in1=R4,
                                    op=OP.mult)
            k150, k151 = 0.25, PK151
        else:
            PK150 = col.tile([P, NCH, 1], F32, tag="PK150")
            nc.vector.scalar_tensor_tensor(out=PK150, in0=r2, scalar=4.0,
                                           in1=SQC, op0=OP.mult, op1=OP.mult)
            k150, k151 = PK150, 1.0
        for j in range(NCH):
            PK = PKs[j]
            if t == "ent":
                nc.vector.memset(PK[:, 150:151], k150)
                nc.vector.tensor_copy(out=PK[:, 151:152], in_=k151[:, j, :])
            else:
                nc.vector.tensor_copy(out=PK[:, 150:151], in_=k150[:, j, :])
                nc.vector.memset(PK[:, 151:152], k151)
            TA = ps.tile([128, P], F16, tag="TT16")
            nc.tensor.transpose(TA, PK[:, 0:128], ident16[:P, :P])
            nc.scalar.copy(out=dA[:, P * j:P * (j + 1)], in_=TA)
            TB = ps.tile([24, P], F16, tag="TT16")
            nc.tensor.transpose(TB, PK[:, 128:152], ident16[:P, :P])
            nc.scalar.copy(out=dB[:, P * j:P * (j + 1)], in_=TB)

    # entity-side rows -> DRAM -> broadcast into replicated tiles.
    # EROWS rows (transpose of PG cols): 0 ws 1 hs 2 areas 3 scr | 4 x0s
    # 5 y0s 6 nx1s 7 ny1s | 8 cx 9 cy.  Rows 0..3 are fp16 broadcast DOUBLED
    # ([P,4,2,NE]) for pair ops; rows 4..7 fp16 broadcast single ([P,4,NE],
    # per-map in0 of the intersection ts ops); rows 8..9 (centers) stay f32
    # (ACT Abs bias path is cancellation-sensitive).
    EROWS = rep.tile([10, NE], F32, tag="EROWS")
    for j in range(NCH):
        TE = ps.tile([10, P], F32, tag="TT")
        nc.tensor.transpose(TE, PG[:, j, :], ident[:P, :P])
        nc.scalar.copy(out=EROWS[:, P * j:P * (j + 1)], in_=TE)
    EROWS16 = rep.tile([8, NE], F16, tag="EROWS16")
    nc.vector.tensor_copy(out=EROWS16, in_=EROWS[0:8, :])
    ED32 = dr.tile([2, NE], F32, tag="ED32", name="ED32")
    nc.sync.dma_start(out=ED32, in_=EROWS[8:10, :])
    ED16 = dr.tile([8, NE], F16, tag="ED16", name="ED16")
    nc.sync.dma_start(out=ED16, in_=EROWS16)
    RALL = rep.tile([P, 2, NE], F32, tag="RALL", name="RALL")
    _ed = ED32[:, :]
    nc.sync.dma_start(out=RALL, in_=bass.AP(
        tensor=_ed.tensor, offset=_ed.offset,
        ap=[[0, P]] + list(_ed.ap)))
    RD4 = rep.tile([P, 4, 2, NE], F16, tag="RD4", name="RD4")
    _eh = ED16[0:4, :]
    for k in range(2):
        nc.sync.dma_start(out=RD4[:, :, k, :], in_=bass.AP(
            tensor=_eh.tensor, offset=_eh.offset,
            ap=[[0, P]] + list(_eh.ap)))
    RB16 = rep.tile([P, 4, NE], F16, tag="RB16", name="RB16")
    _eb = ED16[4:8, :]
    nc.sync.dma_start(out=RB16, in_=bass.AP(
        tensor=_eb.tensor, offset=_eb.offset,
        ap=[[0, P]] + list(_eb.ap)))
    R = {}
    for k, nm_ in enumerate(["CXR", "CYR"]):
        R[nm_] = RALL[:, k, :]
    for k, nm_ in enumerate(["X0R", "Y0R", "X1N", "Y1N"]):
        R[nm_] = RB16[:, k, :]
    for k, nm_ in enumerate(["WER2", "HER2", "AREAR2", "SCR2"]):
        R[nm_] = RD4[:, k, :, :]
    return dict(R=R, VN=VN, PIX_S=PIX_S, PIX_O=PIX_O, PIXA=PIXA, RALL=RALL,
                RD4=RD4, RB16=RB16,
                RHS_A=RHS_A, RHS_B=RHS_B, LS_A=LS_A, LS_B=LS_B,
                LO_A=LO_A, LO_B=LO_B)


def _maps_image(nc, b, env, ctx, mid_thunk):
    io, pre, col, rep, mm, mp, ps, psd, dr = (env[k] for k in
        ("io", "pre", "col", "rep", "mm", "mp", "ps", "psd", "dr"))
    d2b = env["d2b"]
    out_s, out_o = env["out_s"], env["out_o"]
    R = ctx["R"]
    VN = ctx["VN"]
    PIXA = ctx["PIXA"]
    RALL = ctx["RALL"]
    RD4 = ctx["RD4"]
    RHS_A, RHS_B = ctx["RHS_A"], ctx["RHS_B"]

    # fused sub/obj map pipeline: per chunk j, both maps are computed in
    # [P, 2, NE] pair tiles; ops without map-specific scalars run once over
    # the doubled free dim.
    MAPS = ((ctx["LS_A"], ctx["LS_B"], ctx["PIX_S"], 0, 1, out_s),
            (ctx["LO_A"], ctx["LO_B"], ctx["PIX_O"], 2, 3, out_o))

    def m2(tag, dt=F16):
        return mp.tile([P, 2, NE], dt, tag=tag, name=tag)

    def eng(name):
        return nc.gpsimd if name in POOL_OPS else nc.vector

    def stageA_make(j):
        """Allocate pair-j stage-A tiles; return (tiles, dve_thunks,
        pe_thunks, act_thunks). DVE thunks are ordered so no op reads a tile
        written by the immediately-preceding DVE op (write-to-read stall)."""
        ax_p, ay_p, s_p = m2("ax"), m2("ay"), m2("s")
        dxn = m2("dxn")
        dyn = m2("dyn")
        vx1n = [mp.tile([P, NE], F32, tag="vx1n", name="vx1n")
                for _ in range(2)]
        vy1n = [mp.tile([P, NE], F32, tag="vy1n", name="vy1n")
                for _ in range(2)]
        D2s = [psd.tile([P, NE], F32, tag="D2", name="D2") for _ in range(2)]
        pe, act, dve = [], [], []
        for mi, (lA, lB, PIXR, vxc, vyc, odram) in enumerate(MAPS):
            def _mm(mi=mi, lA=lA, lB=lB):
                nc.tensor.matmul(D2s[mi], lhsT=lA[:, P * j:P * (j + 1)],
                                 rhs=RHS_A, start=True, stop=False)
                nc.tensor.matmul(D2s[mi], lhsT=lB[:, P * j:P * (j + 1)],
                                 rhs=RHS_B, start=False, stop=True)
            pe.append(_mm)
            act.append(lambda mi=mi, vxc=vxc: nc.scalar.activation(
                out=ax_p[:, mi, :], in_=R["CXR"], func=AF.Abs,
                bias=VN[:, j, vxc:vxc + 1]))
            act.append(lambda mi=mi, vyc=vyc: nc.scalar.activation(
                out=ay_p[:, mi, :], in_=R["CYR"], func=AF.Abs,
                bias=VN[:, j, vyc:vyc + 1]))
            dve.append(lambda mi=mi, PIXR=PIXR: nc.vector.tensor_scalar(
                out=vx1n[mi], in0=R["X1N"], scalar1=PIXR[:, j, 2:3],
                scalar2=None, op0=OP.max))
            dve.append(lambda mi=mi, PIXR=PIXR: nc.vector.tensor_scalar(
                out=vy1n[mi], in0=R["Y1N"], scalar1=PIXR[:, j, 3:4],
                scalar2=None, op0=OP.max))
        for mi, (lA, lB, PIXR, vxc, vyc, odram) in enumerate(MAPS):
            dve.append(lambda mi=mi, PIXR=PIXR: nc.vector.scalar_tensor_tensor(
                out=dxn[:, mi, :], in0=R["X0R"], scalar=PIXR[:, j, 0:1],
                in1=vx1n[mi], op0=OP.max, op1=OP.add))
            dve.append(lambda mi=mi, PIXR=PIXR: nc.vector.scalar_tensor_tensor(
                out=dyn[:, mi, :], in0=R["Y0R"], scalar=PIXR[:, j, 1:2],
                in1=vy1n[mi], op0=OP.max, op1=OP.add))
        for mi in range(2):
            act.append(lambda mi=mi: nc.scalar.activation(
                out=s_p[:, mi, :], in_=D2s[mi], func=AF.Sqrt, bias=d2b[:P]))
        return (ax_p, ay_p, s_p, dxn, dyn), dve, pe, act

    def fusedStage(j, stA, filler, tail_thunks):
        ax_p, ay_p, s_p, dxn, dyn = stA

        def F():
            for th in filler:
                th()
                return

        def ts(name, in0, s1, op0, s2=None, op1=None, dt=F16, out=None):
            o = out if out is not None else m2(name, dt)
            tgt = o if out is None else out
            if op1 is None:
                eng(name).tensor_scalar(out=tgt, in0=in0, scalar1=s1,
                                        scalar2=None, op0=op0)
            else:
                eng(name).tensor_scalar(out=tgt, in0=in0, scalar1=s1,
                                        scalar2=s2, op0=op0, op1=op1)
            return o

        def tt(name, in0, in1, op, dt=F16):
            o = m2(name, dt)
            eng(name).tensor_tensor(out=o, in0=in0, in1=in1, op=op)
            return o

        PIX0 = MAPS[0][2]
        PIX1 = MAPS[1][2]
        # ---- fused stage; F() = one stage-A(j+1) DVE op as a gap spacer ----
        i1 = m2("i1")
        nc.vector.scalar_tensor_tensor(                        # relu(dx)*dy
            out=i1, in0=dxn, scalar=0.0, in1=dyn, op0=OP.min, op1=OP.mult)
        d12 = tt("d12", ax_p, ay_p, OP.add)                    # pool
        F()
        I2 = m2("I2")
        w1 = m2("w1")
        h1 = m2("h1")
        ts("I2", i1[:, 0, :], 0.0, OP.max, PIX0[:, j, 6:7], OP.subtract,
           out=I2[:, 0, :])
        ts("w1", dxn[:, 0, :], PIX0[:, j, 4:5], OP.add, out=w1[:, 0, :])
        F()
        ts("I2", i1[:, 1, :], 0.0, OP.max, PIX1[:, j, 6:7], OP.subtract,
           out=I2[:, 1, :])
        ts("w1", dxn[:, 1, :], PIX1[:, j, 4:5], OP.add, out=w1[:, 1, :])
        F()
        U = tt("U", R["AREAR2"], I2, OP.subtract)              # union
        ts("h1", dyn[:, 0, :], PIX0[:, j, 5:6], OP.add, out=h1[:, 0, :])
        ts("h1", dyn[:, 1, :], PIX1[:, j, 5:6], OP.add, out=h1[:, 1, :])
        F()
        mI = m2("mI")
        nc.vector.scalar_tensor_tensor(                        # I - u
            out=mI, in0=i1, scalar=0.0, in1=U, op0=OP.max, op1=OP.subtract)
        sq = m2("sq")
        nc.scalar.activation(out=sq, in_=U, func=AF.Square)    # u^2
        wc = tt("wc", w1, R["WER2"], OP.add)
        F()
        hc = tt("hc", h1, R["HER2"], OP.add)
        F()
        A = tt("A", wc, hc, OP.mult)                           # areac
        s1 = ts("s1", s_p, 1.0, OP.add)
        F()
        prod = tt("prod", A, mI, OP.mult)
        Pn = tt("Pn", U, A, OP.mult)                           # pool
        ds1 = ts("ds1", d12, 1.0, OP.add)
        F()
        N = tt("N", sq, prod, OP.add)                          # numerator
        den = tt("den", s1, ds1, OP.mult)                      # pool
        D3 = tt("D3", Pn, den, OP.mult, dt=F32)                # pool, > 0
        # 1/D3 on ScalarE via exp(-ln(D3)) - keeps the (slow, iterative)
        # DVE reciprocal off the bottleneck engine.
        lg = m2("lg")
        nc.scalar.activation(out=lg, in_=D3, func=AF.Ln)
        r3 = m2("r3", F32)
        nc.scalar.activation(out=r3, in_=lg, func=AF.Exp, scale=-1.0)
        F()
        nm = m2("nm")
        nc.vector.scalar_tensor_tensor(out=nm, in0=N, scalar=0.0,
                                       in1=r3, op0=OP.max, op1=OP.mult)
        for th in tail_thunks:
            th()
        outv = tt("outv", nm, R["SCR2"], OP.mult)              # pool
        for mi, (lA, lB, PIXR, vxc, vyc, odram) in enumerate(MAPS):
            nc.sync.dma_start(out=odram[b, P * j:P * (j + 1), :],
                              in_=outv[:, mi, :])
        for th in filler:
            th()

    # depth-2 software pipeline with fine-grained interleave: pair j+1's
    # independent DVE ops are woven between pair j's dependent ops so the
    # DVE never reads a tile written by its immediately-preceding op.
    stA, dveA, peA, actA = stageA_make(0)
    for th in peA + actA + dveA:
        th()
    for j in range(NCH):
        if j + 1 < NCH:
            nxt, dveN, peN, actN = stageA_make(j + 1)
            for th in peN:
                th()
        else:
            nxt, dveN, actN = None, [], []
        fusedStage(j, stA, iter(dveN), actN)
        stA = nxt
        if j == 0:
            mid_thunk()  # next image prep hides in this image's map slack


class _CompiledKernel:
    """Compiled SPMD executable: jit built once, reusable across calls."""

    def __init__(self, nc, n_cores):
        import jax
        from jax.sharding import Mesh, PartitionSpec
        try:
            from jax.experimental.shard_map import shard_map
        except Exception:
            from jax.shard_map import shard_map
        from concourse import bass2jax
        from concourse.bass2jax import _bass_exec_p, install_neuronx_cc_hook

        install_neuronx_cc_hook()
        self.jax = jax
        self.n_cores = n_cores
        partition_name = (nc.partition_id_tensor.name
                          if nc.partition_id_tensor else None)
        in_names, out_names, out_avals, zero_outs = [], [], [], []
        for alloc in nc.m.functions[0].allocations:
            if not isinstance(alloc, mybir.MemoryLocationSet):
                continue
            name = alloc.memorylocations[0].name
            if alloc.kind == "ExternalInput":
                if name != partition_name:
                    in_names.append(name)
            elif alloc.kind == "ExternalOutput":
                shape = tuple(alloc.tensor_shape)
                dtype = mybir.dt.np(alloc.dtype)
                out_names.append(name)
                out_avals.append(jax.core.ShapedArray(shape, dtype))
                zero_outs.append(np.zeros(shape, dtype))
        self.in_names = in_names
        self.out_names = out_names
        self.out_avals = out_avals
        self.zero_outs = zero_outs
        all_in = in_names + out_names
        if partition_name is not None:
            all_in.append(partition_name)

        def _body(*args):
            operands = list(args)
            if partition_name is not None:
                operands.append(bass2jax.partition_id_tensor())
            return tuple(_bass_exec_p.bind(
                *operands,
                out_avals=tuple(out_avals),
                in_names=tuple(all_in),
                out_names=tuple(out_names),
                lowering_input_output_aliases=(),
                sim_require_finite=True,
                sim_require_nnan=True,
                nc=nc,
            ))

        devices = jax.devices()[:n_cores]
        self._mesh = Mesh(np.asarray(devices), ("core",))
        nin = len(in_names) + len(out_names)
        sm = shard_map(_body, mesh=self._mesh,
                       in_specs=(PartitionSpec("core"),) * nin,
                       out_specs=(PartitionSpec("core"),) * len(out_names),
                       check_rep=False)
        from jax.sharding import NamedSharding
        sh = NamedSharding(self._mesh, PartitionSpec("core"))
        in_abst = []
        for alloc in nc.m.functions[0].allocations:
            if not isinstance(alloc, mybir.MemoryLocationSet):
                continue
            name = alloc.memorylocations[0].name
            if alloc.kind == "ExternalInput" and name in in_names:
                shape = tuple(alloc.tensor_shape)
                in_abst.append(jax.ShapeDtypeStruct(
                    (n_cores * shape[0], *shape[1:]), mybir.dt.np(alloc.dtype),
                    sharding=sh))
        out_abst = [jax.ShapeDtypeStruct((n_cores * z.shape[0], *z.shape[1:]),
                                         z.dtype, sharding=sh)
                    for z in self.zero_outs]
        try:
            from concourse.bass2jax import fast_dispatch_compile
            self._fn = fast_dispatch_compile(
                lambda: jax.jit(sm, keep_unused=True)
                .lower(*in_abst, *out_abst).compile())
        except Exception:
            self._fn = jax.jit(sm, keep_unused=True)

    def run(self, in_maps):
        jax = self.jax
        n = self.n_cores
        per_core = [[np.asarray(m[nm]) for nm in self.in_names]
                    for m in in_maps]
        concat_in = [np.concatenate([per_core[c][i] for c in range(n)], axis=0)
                     for i in range(len(self.in_names))]
        concat_zero = [np.zeros((n * z.shape[0], *z.shape[1:]), z.dtype)
                       for z in self.zero_outs]
        outs = jax.block_until_ready(self._fn(*concat_in, *concat_zero))
        return [
            {nm: np.asarray(outs[i]).reshape(n, *self.out_avals[i].shape)[c]
             for i, nm in enumerate(self.out_names)}
            for c in range(n)
        ]


_CACHE = {}


def _get_nc(reps=1):
    key = ("nc", reps)
    if key not in _CACHE:
        _CACHE[key] = _build(N_IMG, reps=reps)
    return _CACHE[key]


def _get_ck(reps=1):
    key = ("ck", reps)
    if key not in _CACHE:
        _CACHE[key] = _CompiledKernel(_get_nc(reps), N_CORES)
    return _CACHE[key]


def kernel(pred_boxes, pred_logits, pred_rel_obj_logits, pred_rel_sub_logits,
           pred_rel_obj_box, pred_rel_sub_box, pred_rel_vec, target_sizes):
    inp = {
        "pred_boxes": np.ascontiguousarray(pred_boxes, dtype=np.float32),
        "pred_logits": np.ascontiguousarray(pred_logits, dtype=np.float32),
        "pred_rel_obj_logits": np.ascontiguousarray(pred_rel_obj_logits, dtype=np.float32),
        "pred_rel_sub_logits": np.ascontiguousarray(pred_rel_sub_logits, dtype=np.float32),
        "pred_rel_obj_box": np.ascontiguousarray(pred_rel_obj_box, dtype=np.float32),
        "pred_rel_sub_box": np.ascontiguousarray(pred_rel_sub_box, dtype=np.float32),
        "pred_rel_vec": np.ascontiguousarray(pred_rel_vec, dtype=np.float32),
        "target_sizes": np.ascontiguousarray(target_sizes, dtype=np.float32),
    }
    in_maps = [{k: v[c * N_IMG:(c + 1) * N_IMG] for k, v in inp.items()}
               for c in range(N_CORES)]
    res = None
    try:
        res = _get_ck().run(in_maps)
    except Exception:
        import time as _time
        _time.sleep(2.0)
        try:
            res = _get_ck().run(in_maps)
        except Exception:
            r = bass_utils.run_bass_kernel_spmd(
                _get_nc(), in_maps, core_ids=list(range(N_CORES)))
            res = r.results
    sub = np.concatenate([res[c]["out_sub"] for c in range(N_CORES)], axis=0)
    obj = np.concatenate([res[c]["out_obj"] for c in range(N_CORES)], axis=0)
    return np.float32(sub), np.float32(obj)



# revision 23
# speedup vs baseline: 7.2974x; 1.0153x over previous
"""Trainium2 Bass kernel for nn_EntitiesIndexingHeadRuleBased (nms_detection).

kernel(**inputs) takes the FULL batch (B=64) and returns (sub_dist, obj_dist),
each [64, 500, 500] float32, matching the reference semantics:

  out_s[r,e] = relu(N) * score_e / (u*A*(d+1)*(s+1))
  N          = u^2 - A*(u - I)        (algebraic form of clip(giou,0)*u*A)
  u, A, I    = union, enclosing area, intersection (ent_e box vs rel box)
  d          = |vx-cx_e| + |vy-cy_e| ;  s = sqrt(cdist^2 via matmul)

Sharding: pure data parallelism - batch 64 split as 8 images per NeuronCore
across 8 cores (SPMD, one Bass program).

Performance notes (v2):
  * fp16 intermediates: tensor_scalar runs in 4x DVE mode, tensor_tensor in
    2x; scalar_tensor_tensor (1x always) is reserved for the few
    cancellation-sensitive ops which compute in the fp32 ALU and emit
    value-scale fp16 (keeps rel-err ~5e-4 despite fp16 storage).
  * box coords are pre-scaled by 1/256 so all giou quantities fit fp16 range.
  * matmul packs are fp16 (1 cyc/row on PE instead of 4 for fp32).
  * a slice of the per-chunk map ops runs on GPSIMD (Pool) to unload DVE.
  * output is fp16 (halves write traffic); host converts to f32.
"""
import sys
sys.path.insert(0, '/opt/trn_rl_repo')

import numpy as np
import bass_rust
import concourse.bass as bass
import concourse.tile as tile
import concourse.tile as tile_mod
from concourse import mybir
from concourse import bass_utils
from concourse.masks import make_identity
from concourse.tile import TileContext

F32 = mybir.dt.float32
F16 = mybir.dt.float16
AF = mybir.ActivationFunctionType
OP = mybir.AluOpType

B = 64
NE = 500
NR = 500
NC1 = 151
NCL = 150
P = 125
NCH = 4
N_CORES = 8
N_IMG = B // N_CORES

SCALE = 1.0 / 256.0          # box-coordinate prescale for fp16 range
D2_BIAS = 3e-4               # clamp for sqrt(d2) against fp16 rounding

# Which map ops run on GPSIMD (Pool) instead of DVE. Walrus only accepts
# plain TensorTensor (add/mult) there - TensorScalarPtr fails engine check.
POOL_OPS = frozenset({"d12", "outv", "Pn", "D3"})

# ---------------------------------------------------------------------------
# Workarounds for the container's walrus: it rejects instructions carrying
# more than one sync-wait command ("Too many sync wait commands").
# ---------------------------------------------------------------------------

_MAXW = 1


def _patched_drain_and_barrier(self, tick_clock, wait_clock):
    ScopedClock = tile_mod.ScopedClock
    carrier = self.nc.sync.nop(nofuse=True)
    wait_clock.add_sem_waits(carrier.ins,
                             ScopedClock({None: tick_clock.global_clock}))
    si = carrier.ins.sync_info
    waits = list(si.on_wait) if si is not None else []
    if len(waits) > _MAXW:
        carrier.ins.sync_info = bass_rust.SyncInfo(
            on_wait=waits[:_MAXW], on_update=[])
        for i in range(_MAXW, len(waits), _MAXW):
            nop = self.nc.sync.nop(nofuse=True)
            nop.ins.sync_info = bass_rust.SyncInfo(
                on_wait=waits[i:i + _MAXW], on_update=[])
    self.nc.sync.drain()
    self.nc.all_engine_barrier()
    assert self.sems is not None
    popped = self.nc._tile_sem_poison_stack.pop()
    assert popped is self._sem_poison
    self.nc.clear_and_free_semaphores(list(self.sems.allocated().values()))
    self.nc.all_engine_barrier()


TileContext._drain_and_barrier = _patched_drain_and_barrier


def _split_waits(nc, maxw=_MAXW):
    """Hoist excess sync waits onto same-engine NoOps placed just before the
    offending instruction (engine streams execute in order)."""
    for fn in nc.m.functions:
        for blk in fn.blocks:
            newl = []
            changed = False
            for ins in blk.instructions:
                si = ins.sync_info
                waits = list(si.on_wait) if si is not None else []
                if len(waits) > maxw:
                    changed = True
                    carried, rest = waits[:-maxw], waits[-maxw:]
                    for i in range(0, len(carried), maxw):
                        nop = mybir.InstNoOp(
                            name=f"{ins.name}-sw{i}",
                            sync_info=mybir.SyncInfo(
                                on_wait=carried[i:i + maxw], on_update=[]),
                            bass_nofuse=True,
                            engine=ins.engine,
                        )
                        newl.append(nop)
                    ins.sync_info = mybir.SyncInfo(
                        on_wait=rest, on_update=list(si.on_update))
                newl.append(ins)
            if changed:
                blk.instructions = newl


# ---------------------------------------------------------------------------
# Kernel builder
# ---------------------------------------------------------------------------

def _bcast(ap, p):
    """[1,N] DRAM AP -> [p,N] partition-broadcast AP (stride-0 partition)."""
    return bass.AP(tensor=ap.tensor, offset=ap.offset,
                   ap=[[0, p]] + list(ap.ap[1:]))


def _build(n_img, reps=1):
    nc = bass.Bass("TRN2", target_bir_lowering=False)

    pb = nc.dram_tensor("pred_boxes", [n_img, NE, 4], F32, kind="ExternalInput")
    pl = nc.dram_tensor("pred_logits", [n_img, NE, NC1], F32, kind="ExternalInput")
    rol = nc.dram_tensor("pred_rel_obj_logits", [n_img, NR, NC1], F32, kind="ExternalInput")
    rsl = nc.dram_tensor("pred_rel_sub_logits", [n_img, NR, NC1], F32, kind="ExternalInput")
    rob = nc.dram_tensor("pred_rel_obj_box", [n_img, NR, 4], F32, kind="ExternalInput")
    rsb = nc.dram_tensor("pred_rel_sub_box", [n_img, NR, 4], F32, kind="ExternalInput")
    rv = nc.dram_tensor("pred_rel_vec", [n_img, NR, 4], F32, kind="ExternalInput")
    tsz = nc.dram_tensor("target_sizes", [n_img, 2], F32, kind="ExternalInput")
    out_s = nc.dram_tensor("out_sub", [n_img, NR, NE], F16, kind="ExternalOutput")
    out_o = nc.dram_tensor("out_obj", [n_img, NR, NE], F16, kind="ExternalOutput")

    with tile.TileContext(nc) as tc:
        with (
            tc.tile_pool(name="singles", bufs=1) as singles,
            tc.tile_pool(name="io", bufs=3) as io,
            tc.tile_pool(name="pre", bufs=5) as pre,
            tc.tile_pool(name="col", bufs=2) as col,
            tc.tile_pool(name="rep", bufs=2) as rep,
            tc.tile_pool(name="mm", bufs=2) as mm,
            tc.tile_pool(name="mp", bufs=2) as mp,
            tc.tile_pool(name="ps", bufs=2, space="PSUM") as ps,
            tc.tile_pool(name="psd", bufs=4, space="PSUM") as psd,
            tc.tile_pool(name="dr", bufs=2, space="DRAM") as dr,
        ):
            ident = singles.tile([128, 128], F32, tag="ident")
            make_identity(nc, ident)
            ident16 = singles.tile([128, 128], F16, tag="ident16")
            nc.vector.tensor_copy(out=ident16, in_=ident)
            d2b = singles.tile([128, 1], F32, tag="d2b")
            nc.vector.memset(d2b, D2_BIAS)

            # image-level software pipeline: image b+1's prep (softmax,
            # boxes, broadcasts) is emitted in the middle of image b's map
            # stage so its ACT/DVE/DMA work hides in the map stage's slack.
            # reps>1 repeats the whole batch back-to-back (timing variant);
            # the pipeline runs straight through the seam.
            env = locals()
            seq = [i % n_img for i in range(n_img * reps)]
            ctx, phases0 = _prep_image(nc, seq[0], env)
            for ph in phases0:
                ph()
            pend = None
            for k, b in enumerate(seq):
                holder = []
                if k + 1 < len(seq):
                    def mid(bn=seq[k + 1], holder=holder):
                        c, ph = _prep_image(nc, bn, env)
                        holder.append(c)
                        return ph
                else:
                    def mid():
                        return None
                pend = _maps_image(nc, b, env, ctx, mid, prev_tail=pend)
                ctx = holder[0] if holder else None
            if pend is not None:
                pend()  # flush the final chunk's division tail
    _split_waits(nc)
    return nc


def _prep_image(nc, b, env):
    io, pre, col, rep, mm, mp, ps, psd, dr = (env[k] for k in
        ("io", "pre", "col", "rep", "mm", "mp", "ps", "psd", "dr"))
    ident = env["ident"]
    ident16 = env["ident16"]
    d2b = env["d2b"]
    pb, pl, rol, rsl, rob, rsb, rv, tsz = (env[k] for k in
        ("pb", "pl", "rol", "rsl", "rob", "rsb", "rv", "tsz"))
    out_s, out_o = env["out_s"], env["out_o"]

    # image-level scalars: W, H broadcast to all partitions (px and /256)
    WH = col.tile([128, 2], F32, tag="WH")
    nc.sync.dma_start(out=WH, in_=_bcast(tsz[b:b + 1, :], 128))
    Ht = WH[:, 0:1]
    Wt = WH[:, 1:2]
    HtP = WH[:P, 0:1]
    WtP = WH[:P, 1:2]
    Ws = col.tile([128, 1], F32, tag="Ws")
    Hs = col.tile([128, 1], F32, tag="Hs")
    nWs = col.tile([128, 1], F32, tag="nWs")
    nHs = col.tile([128, 1], F32, tag="nHs")
    nc.vector.tensor_scalar(out=Ws, in0=Wt, scalar1=SCALE, scalar2=None,
                            op0=OP.mult)
    nc.vector.tensor_scalar(out=Hs, in0=Ht, scalar1=SCALE, scalar2=None,
                            op0=OP.mult)
    nc.vector.tensor_scalar(out=nWs, in0=Wt, scalar1=-SCALE, scalar2=None,
                            op0=OP.mult)
    nc.vector.tensor_scalar(out=nHs, in0=Ht, scalar1=-SCALE, scalar2=None,
                            op0=OP.mult)

    # --- batched box prep: ent / rs / ro processed in one [125,12,*] pass ---
    # columns of PIXA: 0 x0s 1 y0s 2 nx1s 3 ny1s 4 ws 5 hs 6 areas
    BTA = io.tile([P, 3, NCH, 4], F32, tag="BTA")
    for t, dram in enumerate((pb, rsb, rob)):
        nc.sync.dma_start(out=BTA[:, t, :, :],
                          in_=dram[b].rearrange("(j p) c -> p j c", p=P))
    BTF = BTA[:, :, :, :].rearrange("p t j c -> p (t j) c")
    LO = col.tile([P, 12, 2], F32, tag="LO")
    HIc = col.tile([P, 12, 2], F32, tag="HIc")
    nc.vector.scalar_tensor_tensor(
        out=LO, in0=BTF[:, :, 2:4], scalar=-0.5, in1=BTF[:, :, 0:2],
        op0=OP.mult, op1=OP.add)
    nc.vector.scalar_tensor_tensor(
        out=HIc, in0=BTF[:, :, 2:4], scalar=0.5, in1=BTF[:, :, 0:2],
        op0=OP.mult, op1=OP.add)
    PIXA = col.tile([P, 3, NCH, 7], F32, tag="PIXA")
    PIXF = PIXA[:, :, :, :].rearrange("p t j c -> p (t j) c")
    nc.vector.tensor_scalar(out=PIXF[:, :, 0:1], in0=LO[:, :, 0:1],
                            scalar1=Ws[:P], scalar2=None, op0=OP.mult)
    nc.vector.tensor_scalar(out=PIXF[:, :, 1:2], in0=LO[:, :, 1:2],
                            scalar1=Hs[:P], scalar2=None, op0=OP.mult)
    nc.vector.tensor_scalar(out=PIXF[:, :, 2:3], in0=HIc[:, :, 0:1],
                            scalar1=nWs[:P], scalar2=None, op0=OP.mult)
    nc.vector.tensor_scalar(out=PIXF[:, :, 3:4], in0=HIc[:, :, 1:2],
                            scalar1=nHs[:P], scalar2=None, op0=OP.mult)
    nc.vector.tensor_scalar(out=PIXF[:, :, 4:5], in0=BTF[:, :, 2:3],
                            scalar1=Ws[:P], scalar2=None, op0=OP.mult)
    nc.vector.tensor_scalar(out=PIXF[:, :, 5:6], in0=BTF[:, :, 3:4],
                            scalar1=Hs[:P], scalar2=None, op0=OP.mult)
    nc.vector.tensor_tensor(out=PIXF[:, :, 6:7], in0=PIXF[:, :, 4:5],
                            in1=PIXF[:, :, 5:6], op=OP.mult)
    PIX_E = PIXA[:, 0, :, :]
    PIX_S = PIXA[:, 1, :, :]
    PIX_O = PIXA[:, 2, :, :]

    # PG columns (f32): 0 ws 1 hs 2 areas 3 score | 4 x0s 5 y0s 6 nx1s 7 ny1s
    #                   8 cx_px 9 cy_px   (fp16-bound rows first: partition-0
    #                   aligned reads after the PE transpose)
    PG = pre.tile([P, NCH, 10], F32, tag="PG")
    nc.vector.tensor_copy(out=PG[:, :, 0:3], in_=PIX_E[:, :, 4:7])
    nc.vector.tensor_copy(out=PG[:, :, 4:8], in_=PIX_E[:, :, 0:4])
    nc.vector.tensor_scalar(out=PG[:, :, 8:9], in0=BTA[:, 0, :, 0:1],
                            scalar1=WtP, scalar2=None, op0=OP.mult)
    nc.vector.tensor_scalar(out=PG[:, :, 9:10], in0=BTA[:, 0, :, 1:2],
                            scalar1=HtP, scalar2=None, op0=OP.mult)

    # rel_vec endpoints, negated, px units (ACT bias for |cx - vx|).
    # WH4 = [w,h,w,h] per partition via a reversed-stride broadcast DMA of
    # target_sizes ([h,w] in DRAM); VN = (-rel_vec) * WH4 in one op.
    RVt = io.tile([P, NCH, 4], F32, tag="RVt")
    nc.sync.dma_start(out=RVt, in_=rv[b].rearrange("(j p) c -> p j c", p=P))
    WH4 = col.tile([128, 4], F32, tag="WH4")
    _wh = WH[:, :]
    nc.vector.tensor_copy(out=WH4, in_=bass.AP(
        tensor=_wh.tensor, offset=_wh.offset + 1,
        ap=[list(_wh.ap[0]), [0, 2], [-1, 2]]))
    VN = col.tile([P, NCH, 4], F32, tag="VN")
    _w4 = WH4[:P]
    nc.vector.scalar_tensor_tensor(
        out=VN, in0=RVt, scalar=-1.0,
        in1=bass.AP(tensor=_w4.tensor, offset=_w4.offset,
                    ap=[list(_w4.ap[0]), [0, NCH]] + list(_w4.ap[1:])),
        op0=OP.mult, op1=OP.mult)

    # softmax + fp16 packs + PE transposes into class-major matmul operands.
    # The three logits tensors are deferred into per-tensor phases so the
    # prep's ACT burst spreads over the previous image's chunk iterations
    # instead of queueing ahead of its latency-critical map ACT ops.
    RHS_A = mm.tile([128, NE], F16, tag="RHS_A")
    RHS_B = mm.tile([24, NE], F16, tag="RHS_B")
    LS_A = mm.tile([128, NR], F16, tag="LS_A")
    LS_B = mm.tile([24, NR], F16, tag="LS_B")
    LO_A = mm.tile([128, NR], F16, tag="LO_A")
    LO_B = mm.tile([24, NR], F16, tag="LO_B")
    LTs = {}
    for t, ldram in (("ent", pl), ("rs", rsl), ("ro", rol)):
        LT = io.tile([P, NCH, NC1], F32, tag="LT")
        nc.sync.dma_start(out=LT, in_=ldram[b].rearrange("(j p) c -> p j c",
                                                         p=P))
        LTs[t] = LT

    def pack_tensor(t, dA, dB):
        LT = LTs[t]
        # per-chunk exp/square accumulators land in [P, NCH] column tiles so
        # the tiny scalar algebra runs once per image, not once per chunk
        E4 = pre.tile([P, NCH, NC1], F16, tag="E4")
        SM = col.tile([P, NCH, 1], F32, tag="SM")
        SQC = col.tile([P, NCH, 1], F32, tag="SQC")
        for j in range(NCH):
            nc.scalar.activation(out=E4[:, j, :], in_=LT[:, j, :],
                                 func=AF.Exp, accum_out=SM[:, j, :])
            SQ = pre.tile([P, NCL], F16, tag="SQ")
            nc.scalar.activation(out=SQ, in_=E4[:, j, :NCL], func=AF.Square,
                                 accum_out=SQC[:, j, :])
        R4 = col.tile([P, NCH, 1], F32, tag="R4")
        nc.vector.reciprocal(R4, SM)
        pk_s2 = None if t == "ent" else -2.0
        PKs = []
        for j in range(NCH):
            PK = pre.tile([P, 152], F16, tag="PK")
            PKs.append(PK)
            if pk_s2 is None:
                nc.vector.tensor_scalar(out=PK[:, 0:NCL], in0=E4[:, j, :NCL],
                                        scalar1=R4[:, j, :], scalar2=None,
                                        op0=OP.mult)
            else:
                nc.vector.tensor_scalar(out=PK[:, 0:NCL], in0=E4[:, j, :NCL],
                                        scalar1=R4[:, j, :], scalar2=pk_s2,
                                        op0=OP.mult, op1=OP.mult)
        r2 = col.tile([P, NCH, 1], F32, tag="r2")
        nc.vector.tensor_tensor(out=r2, in0=R4, in1=R4, op=OP.mult)
        if t == "ent":
            MX = col.tile([P, NCH, 1], F32, tag="MX")
            nc.vector.tensor_reduce(out=MX, in_=E4[:, :, :NCL],
                                    axis=mybir.AxisListType.X, op=OP.max)
            PK151 = col.tile([P, NCH, 1], F32, tag="PK151")
            nc.vector.tensor_tensor(out=PK151, in0=SQC, in1=r2, op=OP.mult)
            nc.vector.tensor_tensor(out=PG[:, :, 3:4], in0=MX, in1=R4,
                                    op=OP.mult)
            k150, k151 = 0.25, PK151
        else:
            PK150 = col.tile([P, NCH, 1], F32, tag="PK150")
            nc.vector.scalar_tensor_tensor(out=PK150, in0=r2, scalar=4.0,
                                           in1=SQC, op0=OP.mult, op1=OP.mult)
            k150, k151 = PK150, 1.0
        for j in range(NCH):
            PK = PKs[j]
            if t == "ent":
                nc.vector.memset(PK[:, 150:151], k150)
                nc.vector.tensor_copy(out=PK[:, 151:152], in_=k151[:, j, :])
            else:
                nc.vector.tensor_copy(out=PK[:, 150:151], in_=k150[:, j, :])
                nc.vector.memset(PK[:, 151:152], k151)
            TA = ps.tile([128, P], F16, tag="TT16")
            nc.tensor.transpose(TA, PK[:, 0:128], ident16[:P, :P])
            nc.scalar.copy(out=dA[:, P * j:P * (j + 1)], in_=TA)
            TB = ps.tile([24, P], F16, tag="TT16")
            nc.tensor.transpose(TB, PK[:, 128:152], ident16[:P, :P])
            nc.scalar.copy(out=dB[:, P * j:P * (j + 1)], in_=TB)

    R = {}
    ctx = dict(R=R, VN=VN, PIX_S=PIX_S, PIX_O=PIX_O, PIXA=PIXA,
               RHS_A=RHS_A, RHS_B=RHS_B, LS_A=LS_A, LS_B=LS_B,
               LO_A=LO_A, LO_B=LO_B)

    def phase_ent():
        pack_tensor("ent", RHS_A, RHS_B)
        # entity-side rows -> DRAM -> broadcast into replicated tiles.
        # EROWS rows (transpose of PG cols): 0 ws 1 hs 2 areas 3 scr | 4 x0s
        # 5 y0s 6 nx1s 7 ny1s | 8 cx 9 cy.  Rows 0..3 fp16 broadcast DOUBLED
        # ([P,4,2,NE]) for pair ops; rows 4..7 fp16 broadcast single
        # ([P,4,NE], per-map in0 of the intersection ts ops); rows 8..9
        # (centers) stay f32 (ACT Abs bias path is cancellation-sensitive).
        EROWS = rep.tile([10, NE], F32, tag="EROWS")
        for j in range(NCH):
            TE = ps.tile([10, P], F32, tag="TT")
            nc.tensor.transpose(TE, PG[:, j, :], ident[:P, :P])
            nc.scalar.copy(out=EROWS[:, P * j:P * (j + 1)], in_=TE)
        EROWS16 = rep.tile([8, NE], F16, tag="EROWS16")
        nc.vector.tensor_copy(out=EROWS16, in_=EROWS[0:8, :])
        ED32 = dr.tile([2, NE], F32, tag="ED32", name="ED32")
        nc.sync.dma_start(out=ED32, in_=EROWS[8:10, :])
        ED16 = dr.tile([8, NE], F16, tag="ED16", name="ED16")
        nc.sync.dma_start(out=ED16, in_=EROWS16)
        RALL = rep.tile([P, 2, NE], F32, tag="RALL", name="RALL")
        _ed = ED32[:, :]
        nc.sync.dma_start(out=RALL, in_=bass.AP(
            tensor=_ed.tensor, offset=_ed.offset,
            ap=[[0, P]] + list(_ed.ap)))
        RD4 = rep.tile([P, 4, 2, NE], F16, tag="RD4", name="RD4")
        _eh = ED16[0:4, :]
        for k in range(2):
            nc.sync.dma_start(out=RD4[:, :, k, :], in_=bass.AP(
                tensor=_eh.tensor, offset=_eh.offset,
                ap=[[0, P]] + list(_eh.ap)))
        RB16 = rep.tile([P, 4, NE], F16, tag="RB16", name="RB16")
        _eb = ED16[4:8, :]
        nc.sync.dma_start(out=RB16, in_=bass.AP(
            tensor=_eb.tensor, offset=_eb.offset,
            ap=[[0, P]] + list(_eb.ap)))
        for k, nm_ in enumerate(["CXR", "CYR"]):
            R[nm_] = RALL[:, k, :]
        for k, nm_ in enumerate(["X0R", "Y0R", "X1N", "Y1N"]):
            R[nm_] = RB16[:, k, :]
        for k, nm_ in enumerate(["WER2", "HER2", "AREAR2", "SCR2"]):
            R[nm_] = RD4[:, k, :, :]

    phases = [phase_ent,
              lambda: pack_tensor("rs", LS_A, LS_B),
              lambda: pack_tensor("ro", LO_A, LO_B)]
    return ctx, phases


def _maps_image(nc, b, env, ctx, mid_thunk, prev_tail=None):
    io, pre, col, rep, mm, mp, ps, psd, dr = (env[k] for k in
        ("io", "pre", "col", "rep", "mm", "mp", "ps", "psd", "dr"))
    d2b = env["d2b"]
    out_s, out_o = env["out_s"], env["out_o"]
    R = ctx["R"]
    VN = ctx["VN"]
    PIXA = ctx["PIXA"]
    RHS_A, RHS_B = ctx["RHS_A"], ctx["RHS_B"]

    # fused sub/obj map pipeline: per chunk j, both maps are computed in
    # [P, 2, NE] pair tiles; ops without map-specific scalars run once over
    # the doubled free dim.
    MAPS = ((ctx["LS_A"], ctx["LS_B"], ctx["PIX_S"], 0, 1, out_s),
            (ctx["LO_A"], ctx["LO_B"], ctx["PIX_O"], 2, 3, out_o))

    def m2(tag, dt=F16):
        return mp.tile([P, 2, NE], dt, tag=tag, name=tag)

    def eng(name):
        return nc.gpsimd if name in POOL_OPS else nc.vector

    def stageA_make(j):
        """Allocate pair-j stage-A tiles; return (tiles, dve_thunks,
        pe_thunks, act_thunks). DVE thunks are ordered so no op reads a tile
        written by the immediately-preceding DVE op (write-to-read stall).
        The intersection terms run as fp16 ts (4x) + fp16 tt (2x) instead of
        scalar_tensor_tensor (1x): max(x0e,x0r) and max(-x1e,-x1r) per map,
        then dxn = xN + vx1n."""
        ax_p, ay_p, s_p = m2("ax"), m2("ay"), m2("s")
        dxn = m2("dxn")
        dyn = m2("dyn")
        vx1n = [mp.tile([P, NE], F16, tag="vx1n", name="vx1n")
                for _ in range(2)]
        vy1n = [mp.tile([P, NE], F16, tag="vy1n", name="vy1n")
                for _ in range(2)]
        xN = [mp.tile([P, NE], F16, tag="xN", name="xN") for _ in range(2)]
        yN = [mp.tile([P, NE], F16, tag="yN", name="yN") for _ in range(2)]
        D2s = [psd.tile([P, NE], F32, tag="D2", name="D2") for _ in range(2)]
        pe, act, dve = [], [], []
        for mi, (lA, lB, PIXR, vxc, vyc, odram) in enumerate(MAPS):
            def _mm(mi=mi, lA=lA, lB=lB):
                nc.tensor.matmul(D2s[mi], lhsT=lA[:, P * j:P * (j + 1)],
                                 rhs=RHS_A, start=True, stop=False)
                nc.tensor.matmul(D2s[mi], lhsT=lB[:, P * j:P * (j + 1)],
                                 rhs=RHS_B, start=False, stop=True)
            pe.append(_mm)
            act.append(lambda mi=mi, vxc=vxc: nc.scalar.activation(
                out=ax_p[:, mi, :], in_=R["CXR"], func=AF.Abs,
                bias=VN[:, j, vxc:vxc + 1]))
            act.append(lambda mi=mi, vyc=vyc: nc.scalar.activation(
                out=ay_p[:, mi, :], in_=R["CYR"], func=AF.Abs,
                bias=VN[:, j, vyc:vyc + 1]))
            dve.append(lambda mi=mi, PIXR=PIXR: nc.vector.tensor_scalar(
                out=vx1n[mi], in0=R["X1N"], scalar1=PIXR[:, j, 2:3],
                scalar2=None, op0=OP.max))
            dve.append(lambda mi=mi, PIXR=PIXR: nc.vector.tensor_scalar(
                out=vy1n[mi], in0=R["Y1N"], scalar1=PIXR[:, j, 3:4],
                scalar2=None, op0=OP.max))
            dve.append(lambda mi=mi, PIXR=PIXR: nc.vector.tensor_scalar(
                out=xN[mi], in0=R["X0R"], scalar1=PIXR[:, j, 0:1],
                scalar2=None, op0=OP.max))
            dve.append(lambda mi=mi, PIXR=PIXR: nc.vector.tensor_scalar(
                out=yN[mi], in0=R["Y0R"], scalar1=PIXR[:, j, 1:2],
                scalar2=None, op0=OP.max))
        for mi in range(2):
            dve.append(lambda mi=mi: nc.vector.tensor_tensor(
                out=dxn[:, mi, :], in0=xN[mi], in1=vx1n[mi], op=OP.add))
            dve.append(lambda mi=mi: nc.vector.tensor_tensor(
                out=dyn[:, mi, :], in0=yN[mi], in1=vy1n[mi], op=OP.add))
        for mi in range(2):
            # sqrt first in the ACT stream: it gates the s-path (s1/den/D3)
            # while ax/ay only gate d12 via pool
            act.insert(mi, lambda mi=mi: nc.scalar.activation(
                out=s_p[:, mi, :], in_=D2s[mi], func=AF.Sqrt, bias=d2b[:P]))
        return (ax_p, ay_p, s_p, dxn, dyn), dve, pe, act

    def fusedStage(j, stA, filler, tail_thunks, prev_tail):
        ax_p, ay_p, s_p, dxn, dyn = stA

        def F():
            for th in filler:
                th()
                return

        def ts(name, in0, s1, op0, s2=None, op1=None, dt=F16, out=None):
            o = out if out is not None else m2(name, dt)
            tgt = o if out is None else out
            if op1 is None:
                eng(name).tensor_scalar(out=tgt, in0=in0, scalar1=s1,
                                        scalar2=None, op0=op0)
            else:
                eng(name).tensor_scalar(out=tgt, in0=in0, scalar1=s1,
                                        scalar2=s2, op0=op0, op1=op1)
            return o

        def tt(name, in0, in1, op, dt=F16):
            o = m2(name, dt)
            eng(name).tensor_tensor(out=o, in0=in0, in1=in1, op=op)
            return o

        PIX0 = MAPS[0][2]
        PIX1 = MAPS[1][2]
        # ---- fused stage; F() = one stage-A(j+1) DVE op as a gap spacer ----
        dxc = ts("dxc", dxn, 0.0, OP.min)                      # min(dxn,0)
        d12 = tt("d12", ax_p, ay_p, OP.add)                    # pool
        F()
        i1 = tt("i1", dxc, dyn, OP.mult)                       # relu(dx)*dy
        # rotated pipeline: the previous chunk's division tail (D3, ln, exp,
        # nm, outv, DMA) is emitted here, 3 DVE ops into this chunk's head.
        # Its cross-engine waits park in the wait queues while this chunk's
        # independent ops keep all engines dense.
        if prev_tail is not None:
            prev_tail()
        I = ts("I", i1, 0.0, OP.max)                           # intersection
        F()
        I2 = m2("I2")
        w1 = m2("w1")
        h1 = m2("h1")
        ts("I2", I[:, 0, :], PIX0[:, j, 6:7], OP.subtract, out=I2[:, 0, :])
        ts("w1", dxn[:, 0, :], PIX0[:, j, 4:5], OP.add, out=w1[:, 0, :])
        F()
        ts("I2", I[:, 1, :], PIX1[:, j, 6:7], OP.subtract, out=I2[:, 1, :])
        ts("w1", dxn[:, 1, :], PIX1[:, j, 4:5], OP.add, out=w1[:, 1, :])
        F()
        U = tt("U", R["AREAR2"], I2, OP.subtract)              # union
        ts("h1", dyn[:, 0, :], PIX0[:, j, 5:6], OP.add, out=h1[:, 0, :])
        ts("h1", dyn[:, 1, :], PIX1[:, j, 5:6], OP.add, out=h1[:, 1, :])
        F()
        mI = tt("mI", I, U, OP.subtract)                       # I - u
        sq = m2("sq")
        nc.scalar.activation(out=sq, in_=U, func=AF.Square)    # u^2
        wc = tt("wc", w1, R["WER2"], OP.add)
        F()
        hc = tt("hc", h1, R["HER2"], OP.add)
        F()
        A = tt("A", wc, hc, OP.mult)                           # areac
        s1 = ts("s1", s_p, 1.0, OP.add)
        F()
        prod = tt("prod", A, mI, OP.mult)
        Pn = tt("Pn", U, A, OP.mult)                           # pool
        ds1 = ts("ds1", d12, 1.0, OP.add)
        F()
        N = tt("N", sq, prod, OP.add)                          # numerator
        den = tt("den", s1, ds1, OP.mult)                      # s1*ds1 <= 6e3
        for th in tail_thunks:
            th()
        for th in filler:
            th()

        def tail():
            D3 = tt("D3", Pn, den, OP.mult, dt=F32)            # pool, > 0
            lg = m2("lg")
            nc.scalar.activation(out=lg, in_=D3, func=AF.Ln)
            r3 = m2("r3", F32)
            nc.scalar.activation(out=r3, in_=lg, func=AF.Exp, scale=-1.0)
            nm = m2("nm")
            nc.vector.scalar_tensor_tensor(out=nm, in0=N, scalar=0.0,
                                           in1=r3, op0=OP.max, op1=OP.mult)
            outv = tt("outv", nm, R["SCR2"], OP.mult)          # pool
            for mi, (lA, lB, PIXR, vxc, vyc, odram) in enumerate(MAPS):
                nc.sync.dma_start(out=odram[b, P * j:P * (j + 1), :],
                                  in_=outv[:, mi, :])
        return tail

    # depth-2 software pipeline with fine-grained interleave: pair j+1's
    # independent DVE ops are woven between pair j's dependent ops so the
    # DVE never reads a tile written by its immediately-preceding op.
    # The division tail of each chunk is rotated into the next chunk's head
    # (threaded across image boundaries via prev_tail).
    stA, dveA, peA, actA = stageA_make(0)
    for th in peA + actA + dveA:
        th()
    phases = []
    for j in range(NCH):
        if j + 1 < NCH:
            nxt, dveN, peN, actN = stageA_make(j + 1)
            for th in peN:
                th()
        else:
            nxt, dveN, actN = None, [], []
        prev_tail = fusedStage(j, stA, iter(dveN), actN, prev_tail)
        stA = nxt
        if j == 0:
            # next image's base prep (boxes/VN/logit DMAs); its per-tensor
            # pack phases are spread over the remaining chunk iterations so
            # the ACT burst never queues ahead of critical map ops.
            phases = mid_thunk() or []
        elif j - 1 < len(phases):
            phases[j - 1]()
    for ph in phases[NCH - 1:]:
        ph()
    return prev_tail


class _CompiledKernel:
    """Compiled SPMD executable: jit built once, reusable across calls."""

    def __init__(self, nc, n_cores):
        import jax
        from jax.sharding import Mesh, PartitionSpec
        try:
            from jax.experimental.shard_map import shard_map
        except Exception:
            from jax.shard_map import shard_map
        from concourse import bass2jax
        from concourse.bass2jax import _bass_exec_p, install_neuronx_cc_hook

        install_neuronx_cc_hook()
        self.jax = jax
        self.n_cores = n_cores
        partition_name = (nc.partition_id_tensor.name
                          if nc.partition_id_tensor else None)
        in_names, out_names, out_avals, zero_outs = [], [], [], []
        for alloc in nc.m.functions[0].allocations:
            if not isinstance(alloc, mybir.MemoryLocationSet):
                continue
            name = alloc.memorylocations[0].name
            if alloc.kind == "ExternalInput":
                if name != partition_name:
                    in_names.append(name)
            elif alloc.kind == "ExternalOutput":
                shape = tuple(alloc.tensor_shape)
                dtype = mybir.dt.np(alloc.dtype)
                out_names.append(name)
                out_avals.append(jax.core.ShapedArray(shape, dtype))
                zero_outs.append(np.zeros(shape, dtype))
        self.in_names = in_names
        self.out_names = out_names
        self.out_avals = out_avals
        self.zero_outs = zero_outs
        all_in = in_names + out_names
        if partition_name is not None:
            all_in.append(partition_name)

        def _body(*args):
            operands = list(args)
            if partition_name is not None:
                operands.append(bass2jax.partition_id_tensor())
            return tuple(_bass_exec_p.bind(
                *operands,
                out_avals=tuple(out_avals),
                in_names=tuple(all_in),
                out_names=tuple(out_names),
                lowering_input_output_aliases=(),
                sim_require_finite=True,
                sim_require_nnan=True,
                nc=nc,
            ))

        devices = jax.devices()[:n_cores]
        self._mesh = Mesh(np.asarray(devices), ("core",))
        nin = len(in_names) + len(out_names)
        sm = shard_map(_body, mesh=self._mesh,
                       in_specs=(PartitionSpec("core"),) * nin,
                       out_specs=(PartitionSpec("core"),) * len(out_names),
                       check_rep=False)
        from jax.sharding import NamedSharding
        sh = NamedSharding(self._mesh, PartitionSpec("core"))
        in_abst = []
        for alloc in nc.m.functions[0].allocations:
            if not isinstance(alloc, mybir.MemoryLocationSet):
                continue
            name = alloc.memorylocations[0].name
            if alloc.kind == "ExternalInput" and name in in_names:
                shape = tuple(alloc.tensor_shape)
                in_abst.append(jax.ShapeDtypeStruct(
                    (n_cores * shape[0], *shape[1:]), mybir.dt.np(alloc.dtype),
                    sharding=sh))
        out_abst = [jax.ShapeDtypeStruct((n_cores * z.shape[0], *z.shape[1:]),
                                         z.dtype, sharding=sh)
                    for z in self.zero_outs]
        try:
            from concourse.bass2jax import fast_dispatch_compile
            self._fn = fast_dispatch_compile(
                lambda: jax.jit(sm, keep_unused=True)
                .lower(*in_abst, *out_abst).compile())
        except Exception:
            self._fn = jax.jit(sm, keep_unused=True)

    def run(self, in_maps):
        jax = self.jax
        n = self.n_cores
        per_core = [[np.asarray(m[nm]) for nm in self.in_names]
                    for m in in_maps]
        concat_in = [np.concatenate([per_core[c][i] for c in range(n)], axis=0)
                     for i in range(len(self.in_names))]
        concat_zero = [np.zeros((n * z.shape[0], *z.shape[1:]), z.dtype)
                       for z in self.zero_outs]
        outs = jax.block_until_ready(self._fn(*concat_in, *concat_zero))
        return [
            {nm: np.asarray(outs[i]).reshape(n, *self.out_avals[i].shape)[c]
             for i, nm in enumerate(self.out_names)}
            for c in range(n)
        ]


_CACHE = {}


def _get_nc(reps=1):
    key = ("nc", reps)
    if key not in _CACHE:
        _CACHE[key] = _build(N_IMG, reps=reps)
    return _CACHE[key]


def _get_ck(reps=1):
    key = ("ck", reps)
    if key not in _CACHE:
        _CACHE[key] = _CompiledKernel(_get_nc(reps), N_CORES)
    return _CACHE[key]


def kernel(pred_boxes, pred_logits, pred_rel_obj_logits, pred_rel_sub_logits,
           pred_rel_obj_box, pred_rel_sub_box, pred_rel_vec, target_sizes):
    inp = {
        "pred_boxes": np.ascontiguousarray(pred_boxes, dtype=np.float32),
        "pred_logits": np.ascontiguousarray(pred_logits, dtype=np.float32),
        "pred_rel_obj_logits": np.ascontiguousarray(pred_rel_obj_logits, dtype=np.float32),
        "pred_rel_sub_logits": np.ascontiguousarray(pred_rel_sub_logits, dtype=np.float32),
        "pred_rel_obj_box": np.ascontiguousarray(pred_rel_obj_box, dtype=np.float32),
        "pred_rel_sub_box": np.ascontiguousarray(pred_rel_sub_box, dtype=np.float32),
        "pred_rel_vec": np.ascontiguousarray(pred_rel_vec, dtype=np.float32),
        "target_sizes": np.ascontiguousarray(target_sizes, dtype=np.float32),
    }
    in_maps = [{k: v[c * N_IMG:(c + 1) * N_IMG] for k, v in inp.items()}
               for c in range(N_CORES)]
    res = None
    try:
        res = _get_ck().run(in_maps)
    except Exception:
        import time as _time
        _time.sleep(2.0)
        try:
            res = _get_ck().run(in_maps)
        except Exception:
            r = bass_utils.run_bass_kernel_spmd(
                _get_nc(), in_maps, core_ids=list(range(N_CORES)))
            res = r.results
    sub = np.concatenate([res[c]["out_sub"] for c in range(N_CORES)], axis=0)
    obj = np.concatenate([res[c]["out_obj"] for c in range(N_CORES)], axis=0)
    return np.float32(sub), np.float32(obj)

